# revision 82
# baseline (speedup 1.0000x reference)
"""Trainium2 Bass kernel for SSD MultiBox loss (nn_ModelLoss_5970004541458).

Strategy: data-parallel over batch (32 images -> 8 cores x 4 images).
Per core, everything over the prior dim (P=8732, padded to 8960 = 70*128)
runs on-device:
  - jaccard matching (16 boxes x 8960 priors) in bf16 in the LINEAR
    ratio domain: ov = inter * (1/(parea+barea)) (recipA host-precomputed)
    is a monotone bijection of IoU, so max/argmax/threshold semantics
    match IoU with thr 1/3.  Prior rows are pre-expanded across K on the
    host so DVE tensor ops have packed 2-byte operands.
    Forced-assignment sentinels: ovf = max(fmask*102 - 1, ov) -> forced
    elements read 101 (above any real ratio <= 0.5), others unchanged.
  - per-prior one-hot box gather via PE transpose + block-diag matmul (bf16)
  - predicted_scores travel as fp8e4m3 (halves the dominant score DMA;
    ACT exp and the PE U-matmul read fp8 directly; final rel err 1.3e-4)
  - CE: exp on ACT batched across all 4 images per chunk; class sums via
    a bf16 pairwise add tree; per-chunk Ln pipelines ACT behind DVE
  - hard-negative mining via a 2-level 16-way counting grid with bounded-
    error boundary correction; each level's 16 counts come from ONE
    is_gt against a host-provided threshold grid + a bf16 add tree
    (instead of 16 tensor_scalar ops)
  - PSUM->SBUF reduction copies ride the ACT engine; memsets ride Pool;
    independent lps/U-label work fills the two count-allreduce round trips
  - NOTE: rA in fp8 was tried and is SLOWER (mixed fp8xbf16 TT penalty)
    and 4x less accurate; keep rA bf16
All inputs are marshaled host-side into p-major layouts so every DMA is
contiguous per partition.  Each core returns 16 partial sums; the host
combines them into the loss.

This walrus build rejects: gpsimd partition_all_reduce/partition_broadcast,
custom-DVE ops (reciprocal_approx_*), gpsimd TT with broadcast APs or
comparison opcodes, EVENT_SEMAPHORE_RANGE_CLEAR.  Pool (gpsimd) is used
only for dense/strided add/mult/sub/copy/memset.
"""
import sys

for _p in ("/opt/trn_rl_repo",):
    if _p not in sys.path:
        sys.path.insert(0, _p)

import numpy as np

import concourse.bass as bass
import concourse.tile as tile
from concourse import mybir
from concourse.bass_utils import run_bass_kernel_spmd

F32 = mybir.dt.float32
BF16 = mybir.dt.bfloat16
F8 = mybir.dt.float8e4
AX = mybir.AxisListType
OP = mybir.AluOpType
ACTF = mybir.ActivationFunctionType

B, P, C, K = 32, 8732, 81, 16
NCORES = 8
I = B // NCORES          # images per core = 4
PP = 8960                # padded priors = 70 * 128
T = PP // 128            # 70 prior tiles
T8 = 80                  # padded tile count for the m16 max tree
T2 = 72                  # padded tile count for 128-col transpose blocks
NB = T2 * K // 128       # 9 transpose blocks of 128 (t,k)-columns
NCH = 7                  # score chunks (10 tiles each, all 4 images)
CT = T // NCH            # tiles per chunk = 10
THR = 1.0 / 3.0          # ov >= 1/3  <=>  IoU >= 0.5
NQ = 5                   # gathered quantities per box (cx, cy, 5lnw, 5lnh, pad)
TG = 80                  # padded T for the counting grid

# bf16 const blob column offsets (prior rows pre-expanded across K)
B_PX1 = 0
B_PY1 = B_PX1 + T * K
B_PX2 = B_PY1 + T * K
B_PY2 = B_PX2 + T * K
B_TOT = B_PY2 + T * K
# f32 const blob column offsets
C_IV4 = 0
C_PC4 = C_IV4 + T2 * 4
C_ONE = C_PC4 + T2 * 4
C_IDF = C_ONE + 1
C_O128 = C_IDF + 128
C_TOT = C_O128 + 128

_bf16 = np.dtype("uint16")  # bf16 carried as uint16 bit pattern if ml_dtypes absent
try:
    import ml_dtypes

    _bf16 = np.dtype(ml_dtypes.bfloat16)
except ImportError:
    ml_dtypes = None


def _to_bf16(x: np.ndarray) -> np.ndarray:
    if ml_dtypes is not None:
        return x.astype(ml_dtypes.bfloat16)
    u = x.astype(np.float32).view(np.uint32)
    rounded = ((u >> 16) + ((u >> 15) & 1)).astype(np.uint32)
    return (rounded & 0xFFFF).astype(np.uint16)


def _fixup_module(nc: bass.Bass) -> None:
    """Adapt the Tile-generated module to this container's walrus build.

    - EVENT_SEMAPHORE_RANGE_CLEAR is rejected ("ISA wrong length"); the
      preceding Drain(is_reset_sema) already resets the same range, so drop it.
    - Seq-only instructions accept fewer sync waits than Tile emits; hoist
      excess waits onto NoOps placed immediately before (same engine, so
      program order preserves semantics).
    """
    import bass_rust

    for f in nc.m.functions:
        for blk in f.blocks:
            newl = []
            for ins in blk.instructions:
                if getattr(ins, "op_name", None) == "EVENT_SEMAPHORE_RANGE_CLEAR":
                    continue
                si = ins.sync_info
                maxw = 1
                if si is not None and si.on_wait and len(si.on_wait) > maxw:
                    waits = list(si.on_wait)
                    extra, keep = waits[:-maxw], waits[-maxw:]
                    for j in range(0, len(extra), 1):
                        nop = mybir.InstNoOp(
                            name=f"{ins.name}-wsplit{j}", ins=[], outs=[],
                            engine=ins.engine)
                        nop.sync_info = bass_rust.SyncInfo(
                            on_wait=[extra[j]], on_update=[])
                        newl.append(nop)
                    ins.sync_info = bass_rust.SyncInfo(
                        on_wait=keep,
                        on_update=list(si.on_update) if si.on_update else [])
                newl.append(ins)
            blk.instructions = newl


def build_nc(fixup: bool = True) -> bass.Bass:
    nc = bass.Bass()

    # p-major marshaled inputs: [*, 128, free] with contiguous per-partition rows
    d_scores = nc.dram_tensor("scores", [I, 128, T * C], F8, kind="ExternalInput")
    d_locs = nc.dram_tensor("locs", [128, I * T * 4], F32, kind="ExternalInput")
    d_cbf = nc.dram_tensor("cbf", [128, B_TOT], BF16, kind="ExternalInput")
    # counting threshold grid [16, TG]: j/16 (t<T), 999 (pad) -- separate
    # tensor so cbf's readers don't wait for it (tile-granular DMA deps)
    d_ioj = nc.dram_tensor("ioj", [128, 16 * TG], BF16, kind="ExternalInput")
    d_cf32 = nc.dram_tensor("cf32", [128, C_TOT], F32, kind="ExternalInput")
    # all images' box rows (5K each) + onesb row, single partition row
    d_boxall = nc.dram_tensor("boxall", [1, I * 5 * K + 128], BF16,
                              kind="ExternalInput")
    # gather stationaries for all images, p-major
    d_qall = nc.dram_tensor("qall", [128, I * 8 * NQ], BF16,
                            kind="ExternalInput")
    d_lblall = nc.dram_tensor("lblall", [K, I], F32, kind="ExternalInput")
    # 1/(parea + barea) per image, p-major bf16 (host-precomputed)
    d_rA = nc.dram_tensor("rA", [I, 128, T * K], BF16, kind="ExternalInput")
    d_ident = nc.dram_tensor("ident", [128, 128], BF16, kind="ExternalInput")
    d_ones_r = nc.dram_tensor("ones_r", [1, 128], F32, kind="ExternalInput")
    d_iota81 = nc.dram_tensor("iota81", [K, C], F32, kind="ExternalInput")
    # out: per-partition partials; host does the final sums/max.
    # cols: [0:16] scadd (i,4: F, cnt_hi, lps, box), [16:20] ufa (rows 0:K),
    # [20:24] bm, [24:28] np (replicated)
    d_out = nc.dram_tensor("out", [128, 28], F32, kind="ExternalOutput")

    from contextlib import ExitStack

    with tile.TileContext(nc) as tc, ExitStack() as es:
        cpool = es.enter_context(tc.tile_pool(name="consts", bufs=1))
        spool = es.enter_context(tc.tile_pool(name="scores", bufs=1))
        prepool = es.enter_context(tc.tile_pool(name="prein", bufs=1))
        wpool = es.enter_context(tc.tile_pool(name="work", bufs=1))
        epool = es.enter_context(tc.tile_pool(name="exp", bufs=2))
        bpool = es.enter_context(tc.tile_pool(name="batched", bufs=1))
        pp_t = es.enter_context(tc.tile_pool(name="ps_t", bufs=1, space="PSUM"))
        pp_sel = es.enter_context(tc.tile_pool(name="ps_sel", bufs=1, space="PSUM"))
        pp_u = es.enter_context(tc.tile_pool(name="ps_u", bufs=1, space="PSUM"))
        pp_r = es.enter_context(tc.tile_pool(name="ps_r", bufs=2, space="PSUM"))
        rpool = es.enter_context(tc.tile_pool(name="redsb", bufs=4))

        # ------- constants + all per-image loads, DMA-issue spread over
        # ------- SP/ACT queues (engine-synchronous issue serializes)
        cbf = cpool.tile([128, B_TOT], BF16, tag="cbf")
        ioj = cpool.tile([128, 16, TG], BF16, tag="ioj")
        cf32 = cpool.tile([128, C_TOT], F32, tag="cf32")
        ident = cpool.tile([128, 128], BF16, tag="ident")
        ones_r = cpool.tile([1, 128], F32, tag="ones_r")
        iota81 = cpool.tile([K, C], F32, tag="iota81")

        sres_all = spool.tile([128, I, T, C], F8, tag="sres")
        rA_l = [prepool.tile([128, T, K], BF16, tag="rA", bufs=2,
                             name=f"rA{i}") for i in range(I)]
        boxall = cpool.tile([1, I * 5 * K + 128], BF16, tag="boxall")
        qall = cpool.tile([128, I, 8 * NQ], BF16, tag="qall")
        lblall = cpool.tile([K, I], F32, tag="lblall")
        l4all = cpool.tile([128, I, T, 4], F32, tag="l4all")
        boxf_l = [boxall[:, i * 5 * K:(i + 1) * 5 * K]
                  .rearrange("p (a k) -> p a k", k=K) for i in range(I)]
        onesb = boxall[:, I * 5 * K:]
        qblk_l = [qall[:, i, :] for i in range(I)]
        lbl_l = [lblall[:, i:i + 1] for i in range(I)]

        # ACT queue: trigger the ACT_TABLE_LOAD first (it otherwise sits
        # behind the DMA issues and blocks the first bb copy), then rA0
        dwarm = rpool.tile([1, 1], F32, tag="dwarm")
        nc.gpsimd.memset(dwarm[:], 0.0)
        nc.scalar.activation(dwarm[:], dwarm[:], ACTF.Relu)
        nc.scalar.dma_start(out=rA_l[0][:].rearrange("p t k -> p (t k)"),
                            in_=d_rA[0, :, :])
        # SP queue: cbf (the startup long pole) first, then boxall
        nc.sync.dma_start(out=cbf[:], in_=d_cbf[:, :])
        nc.sync.dma_start(out=boxall[:], in_=d_boxall[:, :])
        nc.sync.dma_start(out=cf32[:], in_=d_cf32[:, :])
        nc.sync.dma_start(out=ones_r[:], in_=d_ones_r[:, :])
        nc.sync.dma_start(out=ident[:], in_=d_ident[:, :])
        nc.sync.dma_start(out=rA_l[1][:].rearrange("p t k -> p (t k)"),
                          in_=d_rA[1, :, :])
        nc.sync.dma_start(out=rA_l[2][:].rearrange("p t k -> p (t k)"),
                          in_=d_rA[2, :, :])
        nc.sync.dma_start(out=rA_l[3][:].rearrange("p t k -> p (t k)"),
                          in_=d_rA[3, :, :])
        nc.sync.dma_start(out=qall[:].rearrange("p i q -> p (i q)"),
                          in_=d_qall[:, :])
        nc.sync.dma_start(out=ioj[:].rearrange("p j t -> p (j t)"),
                          in_=d_ioj[:, :])
        nc.sync.dma_start(
            out=sres_all[:, 0].rearrange("p t c -> p (t c)"),
            in_=d_scores[0, :, :])
        nc.sync.dma_start(
            out=sres_all[:, 1].rearrange("p t c -> p (t c)"),
            in_=d_scores[1, :, :])
        nc.sync.dma_start(
            out=sres_all[:, 2].rearrange("p t c -> p (t c)"),
            in_=d_scores[2, :, :])
        nc.sync.dma_start(
            out=sres_all[:, 3].rearrange("p t c -> p (t c)"),
            in_=d_scores[3, :, :])
        nc.sync.dma_start(out=lblall[:], in_=d_lblall[:, :])
        nc.sync.dma_start(out=l4all[:].rearrange("p i t d -> p (i t d)"),
                          in_=d_locs[:, :])
        nc.sync.dma_start(out=iota81[:], in_=d_iota81[:, :])

        def prow(off):  # bf16 pre-expanded prior row view [128, T, K]
            return cbf[:, off:off + T * K].rearrange("p (t k) -> p t k", k=K)

        pxe = {nm: prow(off) for nm, off in
               [("px1", B_PX1), ("py1", B_PY1), ("px2", B_PX2),
                ("py2", B_PY2)]}
        iotaJ = ioj[:]
        iv4 = cf32[:, C_IV4:C_PC4].rearrange("p (t d) -> p t d", d=4)
        pc4 = cf32[:, C_PC4:C_ONE].rearrange("p (t d) -> p t d", d=4)
        ones_p = cf32[:, C_ONE:C_IDF]
        identf = cf32[:, C_IDF:C_O128]
        ones128 = cf32[:, C_O128:C_TOT]

        def rowsum(dst_row_ap, src_ap, n):
            """[P, n] f32 -> [1, n] partition sum written to dst_row_ap."""
            ps = pp_r.tile([128, 128], F32, tag="red_bc")
            nc.tensor.matmul(ps[0:1, :n], lhsT=ones_p[:src_ap.shape[0], :],
                             rhs=src_ap, start=True, stop=True)
            nc.scalar.copy(dst_row_ap, ps[0:1, :n])

        def bcast_row(dst_ap, row_ap, n):
            """[1, n] f32 -> [128, n] replicated (dst may be bf16)."""
            ps = pp_r.tile([128, 128], F32, tag="red_bc")
            nc.tensor.matmul(ps[:, :n], lhsT=ones_r[:], rhs=row_ap,
                             start=True, stop=True)
            nc.scalar.copy(dst_ap, ps[:, :n])

        def allreduce_sum(dst_ap, src_ap, n):
            ps = pp_r.tile([128, 128], F32, tag="red_bc")
            nc.tensor.matmul(ps[:, :n], lhsT=ones128, rhs=src_ap,
                             start=True, stop=True)
            nc.scalar.copy(dst_ap, ps[:, :n])

        def maxreduce_row(dst_row_ap, src_ap, n):
            """[128, n] f32 -> [1, n] partition max written to dst_row_ap."""
            ps = pp_r.tile([128, 128], F32, tag="red_bc")
            nc.tensor.transpose(ps[:n, :], src_ap, identf)
            tsb = rpool.tile([128, 128], F32, tag="red_tsb")
            nc.scalar.copy(tsb[:n, :], ps[:n, :])
            mx = rpool.tile([128, 1], F32, tag="red_mx")
            nc.vector.tensor_reduce(out=mx[:n, :], in_=tsb[:n, :],
                                    axis=AX.X, op=OP.max)
            ps2 = pp_r.tile([128, 128], F32, tag="red_bc")
            nc.tensor.transpose(ps2[0:1, :n], mx[:n, :], identf[:n, :n])
            nc.scalar.copy(dst_row_ap, ps2[0:1, :n])

        # batched buffers [128, I, ...]
        lse4 = bpool.tile([128, I, T], F32, tag="lse4")
        cen4 = bpool.tile([128, I, T], BF16, tag="cen4")
        pos4 = bpool.tile([128, I, T], F32, tag="pos4")
        out_sb = bpool.tile([128, 28], F32, tag="out_sb")
        np4 = out_sb[:, 24:28]
        npt4 = bpool.tile([128, I], F32, tag="npt4")
        k34 = bpool.tile([128, I], F32, tag="k34")
        cnt_all = bpool.tile([128, I, 16], F32, tag="cnt_all")
        cntr = bpool.tile([128, I, 16], F32, tag="cntr")
        lo4 = bpool.tile([128, I], F32, tag="lo4")
        hi4 = bpool.tile([128, I], F32, tag="hi4")
        scadd = out_sb[:, 0:16].rearrange("p (i s) -> p i s", s=4)
        ufa4 = out_sb[0:K, 16:20]
        bm4 = out_sb[:, 20:24]
        # counting grid scratch (level-1 batched across images; level-2
        # reuses per-image slices of the same tiles)
        cen16 = bpool.tile([128, I, TG], BF16, tag="cen16")
        mask1 = bpool.tile([128, I, 16, TG], BF16, tag="mask1")
        m40a = bpool.tile([128, I, 16, 40], BF16, tag="m40a")
        m20a = bpool.tile([128, I, 16, 20], BF16, tag="m20a")
        m10a = bpool.tile([128, I, 16, 10], BF16, tag="m10a")
        m5a = bpool.tile([128, I, 16, 5], BF16, tag="m5a")
        ge4 = bpool.tile([128, I, 16], F32, tag="ge4")
        mc4 = bpool.tile([128, I], F32, tag="mc4")
        fsc4 = bpool.tile([128, I, T], BF16, tag="fsc4")
        lpsb = bpool.tile([128, I, T], F32, tag="lpsb")
        ce0 = bpool.tile([128, I, T], F32, tag="ce0")

        # grid pads: cen16 pad cols stay 0 (grid pad is 999 -> mask 0)
        nc.gpsimd.memset(cen16[:, :, T:], 0.0)

        # ---------------- per-image box rows via PE broadcast -------------
        bb_l = []
        for i in range(I):
            bbt = prepool.tile([128, 5, K], BF16, tag="bb", bufs=4,
                               name=f"bb{i}")
            ps_bb = pp_r.tile([128, 128], F32, tag="red_bc",
                              name=f"psbb{i}")
            nc.tensor.matmul(ps_bb[:, :5 * K], lhsT=onesb,
                             rhs=boxf_l[i].rearrange("p a k -> p (a k)"),
                             start=True, stop=True)
            nc.scalar.copy(bbt[:].rearrange("p a k -> p (a k)"),
                           ps_bb[:, :5 * K])
            bb_l.append(bbt)

        def emit_J(i):
            """Jaccard + per-box max for image i (DVE-heavy, few stalls)."""
            bb = bb_l[i]

            def bcast_b(row):  # [128, K] box row -> [128, T, K] AP (packed k)
                return bb[:, row, :][:, None, :].broadcast_to([128, T, K])

            # ---------------- jaccard (linear ratio domain, bf16) ---------
            ovp80 = wpool.tile([128, T8, K], BF16, tag="ovp80", bufs=2)
            if i < 2:
                nc.gpsimd.memset(ovp80[:, T:, :], -1.0)
            ov = ovp80[:, :T, :]
            ltxy = wpool.tile([128, 2, T, K], BF16, tag="ltxy")
            w0h0 = wpool.tile([128, 2, T, K], BF16, tag="w0h0")
            wrhr = wpool.tile([128, 2, T, K], BF16, tag="wrhr", bufs=2)
            inter = wpool.tile([128, T, K], BF16, tag="inter")

            # x and y stacked on a free dim: max/min/sub are one op each
            px12 = cbf[:, B_PX1:B_PX1 + 2 * T * K].rearrange(
                "p (r t k) -> p r t k", r=2, k=K)
            px34 = cbf[:, B_PX2:B_PX2 + 2 * T * K].rearrange(
                "p (r t k) -> p r t k", r=2, k=K)
            b01 = bb[:, 0:2, :][:, :, None, :].broadcast_to([128, 2, T, K])
            b23 = bb[:, 2:4, :][:, :, None, :].broadcast_to([128, 2, T, K])
            nc.vector.tensor_tensor(out=ltxy[:], in0=px12, in1=b01,
                                    op=OP.max)
            nc.vector.tensor_tensor(out=w0h0[:], in0=px34, in1=b23,
                                    op=OP.min)
            nc.vector.tensor_sub(wrhr[:], w0h0[:], ltxy[:])
            nc.scalar.activation(wrhr[:], wrhr[:], ACTF.Relu)
            nc.vector.tensor_mul(inter[:], wrhr[:, 0], wrhr[:, 1])
            nc.vector.tensor_mul(ov, inter[:], rA_l[i][:])

            # per-box max over priors: dense max tree (80 = 2*2*2*2*5),
            # then the cross-partition max (PE transpose round trip)
            tm1 = wpool.tile([128, 40, K], BF16, tag="tm1")
            nc.vector.tensor_tensor(out=tm1[:], in0=ovp80[:, :40, :],
                                    in1=ovp80[:, 40:, :], op=OP.max)
            tm2 = wpool.tile([128, 20, K], BF16, tag="tm2")
            nc.vector.tensor_tensor(out=tm2[:], in0=tm1[:, :20, :],
                                    in1=tm1[:, 20:, :], op=OP.max)
            tm3 = wpool.tile([128, 10, K], BF16, tag="tm3")
            nc.vector.tensor_tensor(out=tm3[:], in0=tm2[:, :10, :],
                                    in1=tm2[:, 10:, :], op=OP.max)
            tm4 = wpool.tile([128, 5, K], BF16, tag="tm4")
            nc.vector.tensor_tensor(out=tm4[:], in0=tm3[:, :5, :],
                                    in1=tm3[:, 5:, :], op=OP.max)
            m16 = wpool.tile([128, K], F32, tag="m16", bufs=2)
            nc.vector.tensor_reduce(
                out=m16[:], in_=tm4[:].rearrange("p t k -> p k t"),
                axis=AX.X, op=OP.max)
            m16row = wpool.tile([1, K], F32, tag="m16row", bufs=2)
            maxreduce_row(m16row[:], m16[:], K)
            return {"ov": ov, "m16row": m16row}

        def emit_F(i, st):
            """Forcing + one-hot + gather + L1 + U for image i."""
            ov = st["ov"]
            l4 = l4all[:, i]
            qblk = qblk_l[i]
            lbl16 = lbl_l[i]
            sres = sres_all[:, i]

            m16rb = wpool.tile([128, K], BF16, tag="m16rb", bufs=2)
            bcast_row(m16rb[:], st["m16row"][:], K)
            fmask = wpool.tile([128, T, K], BF16, tag="fmask")
            nc.vector.tensor_tensor(
                out=fmask[:], in0=ov,
                in1=m16rb[:][:, None, :].broadcast_to([128, T, K]),
                op=OP.is_equal)
            # uniform sentinel 101 (multi-forced priors go multi-hot; rare
            # and bounded): fm2 = fmask*102 - 1 in {-1, 101}
            ovf = wpool.tile([128, T, K], BF16, tag="ovf", bufs=2)
            fm2 = wpool.tile([128, T, K], BF16, tag="fm2")
            nc.vector.tensor_scalar(out=fm2[:], in0=fmask[:],
                                    scalar1=102.0, scalar2=-1.0,
                                    op0=OP.mult, op1=OP.add)
            nc.vector.tensor_tensor(out=ovf[:], in0=fm2[:], in1=ov,
                                    op=OP.max)
            # per-prior max over k: dense tree on the packed innermost dim
            # per-prior max over k: dense tree on the packed innermost dim
            ms1 = wpool.tile([128, T, 8], BF16, tag="ms1")
            nc.vector.tensor_tensor(out=ms1[:], in0=ovf[:, :, 0:8],
                                    in1=ovf[:, :, 8:16], op=OP.max)
            ms2 = wpool.tile([128, T, 4], BF16, tag="ms2")
            nc.vector.tensor_tensor(out=ms2[:], in0=ms1[:, :, 0:4],
                                    in1=ms1[:, :, 4:8], op=OP.max)
            ms3 = wpool.tile([128, T, 2], BF16, tag="ms3")
            nc.vector.tensor_tensor(out=ms3[:], in0=ms2[:, :, 0:2],
                                    in1=ms2[:, :, 2:4], op=OP.max)
            pm = wpool.tile([128, T], BF16, tag="pm")
            nc.vector.tensor_tensor(out=pm[:], in0=ms3[:, :, 0],
                                    in1=ms3[:, :, 1], op=OP.max)
            # pmz = pm where positive else pm+1 (matches nothing): fuses the
            # one-hot and the pos mask into a single is_eq
            pmz = wpool.tile([128, T], BF16, tag="pmz")
            nc.vector.scalar_tensor_tensor(
                out=pmz[:], in0=pm[:], scalar=THR, in1=pm[:],
                op0=OP.is_lt, op1=OP.add)
            # expand pmz across k on ACT so the is_eq runs packed on DVE
            pmze = wpool.tile([128, T, K], BF16, tag="pmze", bufs=2)
            nc.scalar.copy(
                pmze[:], pmz[:][:, :, None].broadcast_to([128, T, K]))
            wm72 = wpool.tile([128, T2 * K], BF16, tag="wm72", bufs=2)
            if i < 2:
                nc.gpsimd.memset(wm72[:, T * K:], 0.0)
            wmat = wm72[:, :T * K].rearrange("p (t k) -> p t k", k=K)
            nc.vector.tensor_tensor(out=wmat, in0=ovf[:], in1=pmze[:],
                                    op=OP.is_equal)
            nc.vector.tensor_scalar(out=pos4[:, i, :], in0=pm[:],
                                    scalar1=THR, scalar2=None,
                                    op0=OP.is_ge, op1=OP.add,
                                    accum_out=npt4[:, i:i + 1])

            # ---------------- box gather via PE ----------------
            ohT_ps = pp_t.tile([128, NB, 128], BF16, tag="ohT")
            for b in range(NB):
                nc.tensor.transpose(
                    ohT_ps[:, b, :],
                    wm72[:, b * 128:(b + 1) * 128],
                    ident[:])
            ohT_sb = wpool.tile([128, NB * 128], BF16, tag="ohT_sb", bufs=2)
            nc.scalar.copy(ohT_sb[:], ohT_ps[:].rearrange("p b n -> p (b n)"))

            sel_ps = pp_sel.tile([8 * NQ, NB, 128], F32, tag="sel")
            for b in range(NB):
                nc.tensor.matmul(sel_ps[:, b, :], lhsT=qblk,
                                 rhs=ohT_sb[:, b * 128:(b + 1) * 128],
                                 start=True, stop=True)
            sel_sb = wpool.tile([8 * NQ, NB * 128], BF16, tag="sel_sb",
                                bufs=2)
            nc.scalar.copy(sel_sb[:], sel_ps[:].rearrange("p b n -> p (b n)"))
            bk_ps = pp_t.tile([128, NB, 8 * NQ], BF16, tag="ohT")
            for b in range(NB):
                nc.tensor.transpose(
                    bk_ps[:, b, :],
                    sel_sb[:, b * 128:(b + 1) * 128],
                    ident[:8 * NQ, :8 * NQ])
            selq = wpool.tile([128, NB * 8 * NQ], BF16, tag="selq", bufs=2)
            nc.scalar.copy(selq[:], bk_ps[:].rearrange("p b n -> p (b n)"))
            # selq[p, (blk*40 + tb*5 + q)] = sel_q at t = blk*8+tb
            sel4 = selq[:].rearrange("p (t q) -> p t q", q=NQ)[:, :, 0:4]

            # ---------------- box L1 (Pool chain + ACT abs-accum) ---------
            lp4 = wpool.tile([128, T, 4], F32, tag="lp4")
            nc.gpsimd.tensor_add(lp4[:], l4, pc4[:, :T, :])
            tb1 = wpool.tile([128, T, 4], F32, tag="tb1")
            nc.gpsimd.tensor_mul(tb1[:], sel4[:, :T, :], iv4[:, :T, :])
            nc.gpsimd.tensor_sub(tb1[:], lp4[:], tb1[:])
            nc.vector.tensor_tensor(
                out=tb1[:], in0=tb1[:],
                in1=pos4[:, i, :][:, :, None].broadcast_to([128, T, 4]),
                op=OP.mult)
            nc.scalar.activation(tb1[:], tb1[:], ACTF.Abs,
                                 accum_out=scadd[:, i, 3:4])

            # ---------------- U matrix (score at label) ----------------
            u_ps = pp_u.tile([K, C], F32, tag="u")
            for t_ in range(T):
                nc.tensor.matmul(u_ps[:], lhsT=wmat[:, t_, :],
                                 rhs=sres[:, t_, :],
                                 start=(t_ == 0), stop=(t_ == T - 1))
            u_sb = wpool.tile([K, C], F32, tag="u_sb", bufs=4)
            nc.scalar.copy(u_sb[:], u_ps[:])
            u_sb_l.append(u_sb)

        # sequential per image: higher cross-engine concurrency (software
        # pipelining J/F) measured ~20% slower per-op from SBUF contention
        u_sb_l = []
        for i in range(I):
            emit_F(i, emit_J(i))

        # ------- CE: exp (ACT) + class sums (bf16 DVE add tree),
        # ------- batched across all I images per chunk -----------------
        for ch in range(NCH):
            et = epool.tile([128, I, CT, C], BF16, tag="et", bufs=3)
            nc.scalar.activation(
                et[:], sres_all[:, :, ch * CT:(ch + 1) * CT, :], ACTF.Exp)
            e3 = et[:].rearrange("p i t c -> p (i t) c")
            t40 = epool.tile([128, I * CT, 40], BF16, tag="t40")
            t20 = epool.tile([128, I * CT, 20], BF16, tag="t20")
            t10 = epool.tile([128, I * CT, 10], BF16, tag="t10")
            t5 = epool.tile([128, I * CT, 5], BF16, tag="t5")
            secc = epool.tile([128, I, CT], F32, tag="secc")
            with nc.allow_low_precision("bf16 class sums"):
                nc.vector.tensor_add(t40[:], e3[:, :, 0:40], e3[:, :, 40:80])
                nc.vector.tensor_add(t20[:], t40[:, :, 0:20],
                                     t40[:, :, 20:40])
                nc.vector.tensor_add(t10[:], t20[:, :, 0:10],
                                     t20[:, :, 10:20])
                nc.vector.tensor_add(t5[:], t10[:, :, 0:5], t10[:, :, 5:10])
            nc.vector.tensor_reduce(
                out=secc[:].rearrange("p i t -> p (i t)"), in_=t5[:],
                axis=AX.X, op=OP.add)
            nc.vector.tensor_add(secc[:], secc[:], et[:, :, :, 80])
            nc.scalar.activation(
                lse4[:, :, ch * CT:(ch + 1) * CT], secc[:], ACTF.Ln)

        # ce0/cen/lps batched
        nc.vector.tensor_sub(ce0[:], lse4[:], sres_all[:, :, :, 0])
        nc.vector.scalar_tensor_tensor(
            out=cen4[:], in0=pos4[:], scalar=0.5,
            in1=ce0[:], op0=OP.is_lt, op1=OP.mult)

        # n_pos allreduce + k3, batched
        allreduce_sum(np4[:], npt4[:], I)
        nc.vector.tensor_scalar(out=k34[:], in0=np4[:], scalar1=3.0,
                                scalar2=None, op0=OP.mult)

        # ---- counting level 1, batched: 16 counts per image via one
        # ---- is_gt against the j/16 grid (cen/16 is exact in bf16)
        nc.vector.tensor_scalar(out=cen16[:, :, :T], in0=cen4[:],
                                scalar1=1.0 / 16, scalar2=None, op0=OP.mult)
        nc.vector.tensor_tensor(
            out=mask1[:],
            in0=cen16[:, :, None, :].broadcast_to([128, I, 16, TG]),
            in1=iotaJ[:, None, :, :].broadcast_to([128, I, 16, TG]),
            op=OP.is_gt)
        with nc.allow_low_precision("bf16 count sums"):
            nc.vector.tensor_add(m40a[:], mask1[:, :, :, 0:40],
                                 mask1[:, :, :, 40:80])
            nc.vector.tensor_add(m20a[:], m40a[:, :, :, 0:20],
                                 m40a[:, :, :, 20:40])
            nc.vector.tensor_add(m10a[:], m20a[:, :, :, 0:10],
                                 m20a[:, :, :, 10:20])
            nc.vector.tensor_add(m5a[:], m10a[:, :, :, 0:5],
                                 m10a[:, :, :, 5:10])
        nc.vector.tensor_reduce(
            out=cnt_all[:].rearrange("p i j -> p (i j)"), in_=m5a[:],
            axis=AX.X, op=OP.add)
        allreduce_sum(cntr[:].rearrange("p i j -> p (i j)"),
                      cnt_all[:].rearrange("p i j -> p (i j)"), I * 16)

        # fill the count-allreduce round trip: lps partial + U-label pick
        nc.vector.tensor_mul(lpsb[:], pos4[:], lse4[:])
        for i in range(I):
            ufx = wpool.tile([K, C], F32, tag="ufx")
            nc.vector.scalar_tensor_tensor(
                out=ufx[:], in0=iota81[:], scalar=lbl_l[i], in1=u_sb_l[i][:],
                op0=OP.is_equal, op1=OP.mult, accum_out=ufa4[:, i:i + 1])

        # ---------------- mining: lo per image, then level-2 batched -----
        for i in range(I):
            # lo = (#edges with count >= k) - 1   (edges j = 0..15)
            nc.vector.tensor_scalar(out=ge4[:, i, :], in0=cntr[:, i, :],
                                    scalar1=k34[:, i:i + 1], scalar2=None,
                                    op0=OP.is_ge, op1=OP.add,
                                    accum_out=lo4[:, i:i + 1])
        nc.vector.tensor_scalar(out=lo4[:], in0=lo4[:],
                                scalar1=-1.0, scalar2=None, op0=OP.add)
        # level 2: thresholds lo + j/16 via (cen - lo) > j/16, all images
        for i in range(I):
            nc.vector.tensor_scalar(out=cen16[:, i, :T],
                                    in0=cen4[:, i, :],
                                    scalar1=lo4[:, i:i + 1],
                                    scalar2=None, op0=OP.subtract)
        nc.vector.tensor_tensor(
            out=mask1[:],
            in0=cen16[:, :, None, :].broadcast_to([128, I, 16, TG]),
            in1=iotaJ[:, None, :, :].broadcast_to([128, I, 16, TG]),
            op=OP.is_gt)
        with nc.allow_low_precision("bf16 count sums"):
            nc.vector.tensor_add(m40a[:], mask1[:, :, :, 0:40],
                                 mask1[:, :, :, 40:80])
            nc.vector.tensor_add(m20a[:], m40a[:, :, :, 0:20],
                                 m40a[:, :, :, 20:40])
            nc.vector.tensor_add(m10a[:], m20a[:, :, :, 0:10],
                                 m20a[:, :, :, 10:20])
            nc.vector.tensor_add(m5a[:], m10a[:, :, :, 0:5],
                                 m10a[:, :, :, 5:10])
        nc.vector.tensor_reduce(
            out=cnt_all[:].rearrange("p i j -> p (i j)"), in_=m5a[:],
            axis=AX.X, op=OP.add)
        allreduce_sum(cntr[:].rearrange("p i j -> p (i j)"),
                      cnt_all[:].rearrange("p i j -> p (i j)"), I * 16)
        nc.vector.tensor_reduce(out=scadd[:, :, 2], in_=lpsb[:],
                                axis=AX.X, op=OP.add)
        for i in range(I):
            nc.vector.tensor_scalar(out=ge4[:, i, :], in0=cntr[:, i, :],
                                    scalar1=k34[:, i:i + 1], scalar2=None,
                                    op0=OP.is_ge, op1=OP.add,
                                    accum_out=mc4[:, i:i + 1])
        nc.vector.tensor_scalar(out=mc4[:], in0=mc4[:], scalar1=1.0 / 16,
                                scalar2=None, op0=OP.mult)
        nc.vector.tensor_add(hi4[:], mc4[:], lo4[:])
        # F(hi), count(hi), boundary max per image (independent chains)
        for i in range(I):
            nc.vector.scalar_tensor_tensor(
                out=fsc4[:, i, :], in0=cen4[:, i, :], scalar=hi4[:, i:i + 1],
                in1=cen4[:, i, :], op0=OP.is_gt, op1=OP.mult,
                accum_out=scadd[:, i, 0:1])
            nc.vector.tensor_scalar(out=fsc4[:, i, :], in0=cen4[:, i, :],
                                    scalar1=hi4[:, i:i + 1], scalar2=None,
                                    op0=OP.is_gt, op1=OP.add,
                                    accum_out=scadd[:, i, 1:2])
            nc.vector.scalar_tensor_tensor(
                out=fsc4[:, i, :], in0=cen4[:, i, :], scalar=hi4[:, i:i + 1],
                in1=cen4[:, i, :], op0=OP.is_le, op1=OP.mult)
            nc.vector.tensor_reduce(out=bm4[:, i:i + 1], in_=fsc4[:, i, :],
                                    axis=AX.X, op=OP.max)

        nc.sync.dma_start(out=d_out[:, :], in_=out_sb[:])

    if fixup:
        _fixup_module(nc)
    return nc


def prepare_inputs(predicted_locs, predicted_scores, boxes, labels,
                   priors_centers):
    """Shard + marshal the full inputs into 8 per-core in_maps (p-major)."""
    predicted_locs = np.asarray(predicted_locs, np.float32)
    predicted_scores = np.asarray(predicted_scores, np.float32)
    boxes = np.asarray(boxes, np.float32)
    labels_f = np.asarray(labels).astype(np.float32)
    priors = np.asarray(priors_centers, np.float32)

    npad = PP - P
    # scores: pad rows have class0=0, others -50 -> lse=0, S0=0, ce0=0 exactly
    pad_scores = np.full((B, npad, C), -50.0, np.float32)
    pad_scores[:, :, 0] = 0.0
    scores_p = np.concatenate([predicted_scores, pad_scores], axis=1)
    # p-major: [B, 128, T*C]
    scores_pm = np.ascontiguousarray(
        scores_p.reshape(B, T, 128, C).transpose(0, 2, 1, 3)
    ).reshape(B, 128, T * C)
    scores_bf = scores_pm.astype(ml_dtypes.float8_e4m3)
    locs_p = np.concatenate(
        [predicted_locs, np.zeros((B, npad, 4), np.float32)], axis=1)
    locs_pm = np.ascontiguousarray(
        locs_p.reshape(B, T, 128, 4).transpose(0, 2, 1, 3)
    ).reshape(B, 128, T * 4)

    # prior rows pre-expanded across K (p-major, bf16)
    pad_pri = np.tile(np.array([-100.0, -100.0, 1.0, 1.0], np.float32),
                      (npad, 1))
    pri = np.concatenate([priors, pad_pri], axis=0)
    pcx, pcy, pw, ph = pri[:, 0], pri[:, 1], pri[:, 2], pri[:, 3]

    def pm_grid(v):  # [PP] -> [128, T]
        return np.ascontiguousarray(v.astype(np.float32).reshape(T, 128).T)

    def pexp(v):  # [PP] -> [128, T*K] expanded across K
        return np.repeat(pm_grid(v)[:, :, None], K, axis=2).reshape(128, T * K)

    # counting threshold grid [16, TG]: j/16 for t<T, 999 on pads
    ioj = np.full((16, TG), 999.0, np.float32)
    ioj[:, :T] = (np.arange(16, dtype=np.float32) / 16.0)[:, None]
    ioj_b = np.tile(ioj.reshape(1, 16 * TG), (128, 1))
    cbf = _to_bf16(np.concatenate(
        [pexp(pcx - pw / 2), pexp(pcy - ph / 2), pexp(pcx + pw / 2),
         pexp(pcy + ph / 2)], axis=1))
    assert cbf.shape[1] == B_TOT
    ioj_bf = _to_bf16(ioj_b)

    # iv4/pc4 [128, T2, 4] host-assembled (d = x, y, w, h; tail zero)
    iv4 = np.zeros((128, T2, 4), np.float32)
    pc4 = np.zeros((128, T2, 4), np.float32)
    iv4[:, :T, 0] = pm_grid(10.0 / pw)
    iv4[:, :T, 1] = pm_grid(10.0 / ph)
    iv4[:, :T, 2] = 1.0
    iv4[:, :T, 3] = 1.0
    pc4[:, :T, 0] = pm_grid(pcx * (10.0 / pw))
    pc4[:, :T, 1] = pm_grid(pcy * (10.0 / ph))
    pc4[:, :T, 2] = pm_grid(5.0 * np.log(pw))
    pc4[:, :T, 3] = pm_grid(5.0 * np.log(ph))

    ones_p = np.ones((128, 1), np.float32)
    identf = np.eye(128, dtype=np.float32)
    cf32 = np.concatenate(
        [iv4.reshape(128, T2 * 4), pc4.reshape(128, T2 * 4),
         ones_p, identf, np.ones((128, 128), np.float32)],
        axis=1).astype(np.float32)
    assert cf32.shape[1] == C_TOT

    bx1, by1, bx2, by2 = (boxes[:, :, d] for d in range(4))
    barea = (bx2 - bx1) * (by2 - by1)
    boxf = np.stack([bx1, by1, bx2, by2, barea], axis=1)      # [B, 5, K]
    # rA[b, p, t, k] = 1/(parea[p,t] + barea[b,k]) (p-major)
    parea_pm = pm_grid(pw * ph)                               # [128, T]
    rA = _to_bf16(1.0 / (
        parea_pm[None, :, :, None] + barea[:, None, None, :].astype(np.float64)
    ).reshape(B, 128, T * K))
    q5 = np.stack([
        (bx1 + bx2) / 2, (by1 + by2) / 2,
        5.0 * np.log(bx2 - bx1), 5.0 * np.log(by2 - by1),
        np.zeros_like(bx1),
    ], axis=2).astype(np.float32)                           # [B, K, 5]
    qblk_f = np.zeros((B, 128, 8 * NQ), np.float32)
    for tb in range(8):
        qblk_f[:, tb * K:(tb + 1) * K, tb * NQ:(tb + 1) * NQ] = q5

    ident = _to_bf16(np.eye(128, dtype=np.float32))
    ones_r = np.ones((1, 128), np.float32)
    iota81 = np.tile(np.arange(C, dtype=np.float32), (K, 1))

    in_maps = []
    for c in range(NCORES):
        sl = slice(c * I, (c + 1) * I)
        boxall = _to_bf16(np.concatenate(
            [boxf[sl].reshape(-1), np.ones(128, np.float32)])[None, :])
        qall = _to_bf16(np.ascontiguousarray(
            qblk_f[sl].transpose(1, 0, 2)).reshape(128, I * 8 * NQ))
        lblall = np.ascontiguousarray(labels_f[sl].T)         # [K, I]
        locs2 = np.ascontiguousarray(
            locs_pm[sl].transpose(1, 0, 2)).reshape(128, I * T * 4)
        in_maps.append({
            "scores": scores_bf[sl],
            "locs": locs2,
            "cbf": cbf,
            "ioj": ioj_bf,
            "cf32": cf32,
            "boxall": boxall,
            "qall": qall,
            "lblall": lblall,
            "rA": rA[sl],
            "ident": ident,
            "ones_r": ones_r,
            "iota81": iota81,
        })
    return in_maps


def combine_outputs(outs):
    """outs: list of 8 per-core [128, 28] partials -> scalar loss."""
    n_pos_total = 0.0
    box_sum = 0.0
    class_sum = 0.0
    for o in outs:
        o = np.asarray(o, np.float64)
        sc = o[:, :16].reshape(128, I, 4).sum(axis=0)   # [I, 4]
        uf = o[:K, 16:20].sum(axis=0)                   # [I]
        bm = o[:, 20:24].max(axis=0)                    # [I]
        npi = o[0, 24:28]                               # [I]
        f_sum, cnt_hi, lps, box = sc[:, 0], sc[:, 1], sc[:, 2], sc[:, 3]
        r = (3.0 * npi - cnt_hi) * bm + f_sum
        cep = lps - uf
        n_pos_total += npi.sum()
        box_sum += box.sum()
        class_sum += (cep + r).sum()
    loss = class_sum / n_pos_total + box_sum / (n_pos_total * 4.0)
    return np.float32(loss)


_NC_CACHE = {}


def kernel(predicted_locs, predicted_scores, boxes, labels, priors_centers):
    if "nc" not in _NC_CACHE:
        _NC_CACHE["nc"] = build_nc()
    nc = _NC_CACHE["nc"]
    in_maps = prepare_inputs(predicted_locs, predicted_scores, boxes, labels,
                             priors_centers)
    res = run_bass_kernel_spmd(nc, in_maps, list(range(NCORES)))
    outs = [res.results[c]["out"] for c in range(NCORES)]
    return combine_outputs(outs)


if __name__ == "__main__":
    import reference as R

    inputs = {k: np.asarray(v) for k, v in R.setup_inputs().items()}
    print("loss =", kernel(**inputs))


# revision 83
# speedup vs baseline: 1.1154x; 1.1154x over previous
"""Trainium2 Bass kernel for SSD MultiBox loss (nn_ModelLoss_5970004541458).

Strategy: data-parallel over batch (32 images -> 8 cores x 4 images).
Per core, everything over the prior dim (P=8732, padded to 8960 = 70*128)
runs on-device:
  - jaccard matching (16 boxes x 8960 priors) in bf16 in the LINEAR
    ratio domain: ov = inter * (1/(parea+barea)) (recipA host-precomputed)
    is a monotone bijection of IoU, so max/argmax/threshold semantics
    match IoU with thr 1/3.  Prior rows are pre-expanded across K on the
    host so DVE tensor ops have packed 2-byte operands.
    Forced-assignment sentinels: ovf = max(fmask*102 - 1, ov) -> forced
    elements read 101 (above any real ratio <= 0.5), others unchanged.
  - per-prior one-hot box gather via PE transpose + block-diag matmul (bf16)
  - CE: exp on ACT batched across all 4 images per chunk; class sums via
    a bf16 pairwise add tree at DVE 2x mode
  - hard-negative mining via a 2-level 16-way counting grid with bounded-
    error boundary correction; each level's 16 counts come from ONE
    is_gt against a host-provided threshold grid + a bf16 add tree
    (instead of 16 tensor_scalar ops)
  - PSUM->SBUF reduction copies ride the ACT engine; memsets ride Pool
All inputs are marshaled host-side into p-major layouts so every DMA is
contiguous per partition.  Each core returns 16 partial sums; the host
combines them into the loss.

This walrus build rejects: gpsimd partition_all_reduce/partition_broadcast,
custom-DVE ops (reciprocal_approx_*), gpsimd TT with broadcast APs or
comparison opcodes, EVENT_SEMAPHORE_RANGE_CLEAR.  Pool (gpsimd) is used
only for dense/strided add/mult/sub/copy/memset.
"""
import sys

for _p in ("/opt/trn_rl_repo",):
    if _p not in sys.path:
        sys.path.insert(0, _p)

import numpy as np

import concourse.bass as bass
import concourse.tile as tile
from concourse import mybir
from concourse.bass_utils import run_bass_kernel_spmd

F32 = mybir.dt.float32
BF16 = mybir.dt.bfloat16
F8 = mybir.dt.float8e4
AX = mybir.AxisListType
OP = mybir.AluOpType
ACTF = mybir.ActivationFunctionType

B, P, C, K = 32, 8732, 81, 16
NCORES = 8
I = B // NCORES          # images per core = 4
PP = 8960                # padded priors = 70 * 128
T = PP // 128            # 70 prior tiles
T8 = 80                  # padded tile count for the m16 max tree
T2 = 72                  # padded tile count for 128-col transpose blocks
NB = T2 * K // 128       # 9 transpose blocks of 128 (t,k)-columns
NCH = 7                  # score chunks (10 tiles each, all 4 images)
CT = T // NCH            # tiles per chunk = 10
THR = 1.0 / 3.0          # ov >= 1/3  <=>  IoU >= 0.5
NQ = 5                   # gathered quantities per box (cx, cy, 5lnw, 5lnh, pad)
TG = 80                  # padded T for the counting grid

# bf16 const blob column offsets (prior rows pre-expanded across K)
B_PX1 = 0
B_PY1 = B_PX1 + T * K
B_PX2 = B_PY1 + T * K
B_PY2 = B_PX2 + T * K
B_TOT = B_PY2 + T * K
# f32 const blob column offsets
C_IV4 = 0
C_PC4 = C_IV4 + T2 * 4
C_ONE = C_PC4 + T2 * 4
C_IDF = C_ONE + 1
C_O128 = C_IDF + 128
C_TOT = C_O128 + 128

_bf16 = np.dtype("uint16")  # bf16 carried as uint16 bit pattern if ml_dtypes absent
try:
    import ml_dtypes

    _bf16 = np.dtype(ml_dtypes.bfloat16)
except ImportError:
    ml_dtypes = None


def _to_bf16(x: np.ndarray) -> np.ndarray:
    if ml_dtypes is not None:
        return x.astype(ml_dtypes.bfloat16)
    u = x.astype(np.float32).view(np.uint32)
    rounded = ((u >> 16) + ((u >> 15) & 1)).astype(np.uint32)
    return (rounded & 0xFFFF).astype(np.uint16)


def _fixup_module(nc: bass.Bass) -> None:
    """Adapt the Tile-generated module to this container's walrus build.

    - EVENT_SEMAPHORE_RANGE_CLEAR is rejected ("ISA wrong length"); the
      preceding Drain(is_reset_sema) already resets the same range, so drop it.
    - Seq-only instructions accept fewer sync waits than Tile emits; hoist
      excess waits onto NoOps placed immediately before (same engine, so
      program order preserves semantics).
    """
    import bass_rust

    for f in nc.m.functions:
        for blk in f.blocks:
            newl = []
            for ins in blk.instructions:
                if getattr(ins, "op_name", None) == "EVENT_SEMAPHORE_RANGE_CLEAR":
                    continue
                si = ins.sync_info
                maxw = 1
                if si is not None and si.on_wait and len(si.on_wait) > maxw:
                    waits = list(si.on_wait)
                    extra, keep = waits[:-maxw], waits[-maxw:]
                    for j in range(0, len(extra), 1):
                        nop = mybir.InstNoOp(
                            name=f"{ins.name}-wsplit{j}", ins=[], outs=[],
                            engine=ins.engine)
                        nop.sync_info = bass_rust.SyncInfo(
                            on_wait=[extra[j]], on_update=[])
                        newl.append(nop)
                    ins.sync_info = bass_rust.SyncInfo(
                        on_wait=keep,
                        on_update=list(si.on_update) if si.on_update else [])
                newl.append(ins)
            blk.instructions = newl


def build_nc(fixup: bool = True) -> bass.Bass:
    nc = bass.Bass()

    # p-major marshaled inputs: [*, 128, free] with contiguous per-partition rows
    d_scores = nc.dram_tensor("scores", [I, 128, T * C], F8, kind="ExternalInput")
    d_locs = nc.dram_tensor("locs", [128, I * T * 4], F32, kind="ExternalInput")
    d_cbf = nc.dram_tensor("cbf", [128, B_TOT], BF16, kind="ExternalInput")
    # counting threshold grid [16, TG]: j/16 (t<T), 999 (pad) -- separate
    # tensor so cbf's readers don't wait for it (tile-granular DMA deps)
    d_ioj = nc.dram_tensor("ioj", [128, 16 * TG], BF16, kind="ExternalInput")
    d_cf32 = nc.dram_tensor("cf32", [128, C_TOT], F32, kind="ExternalInput")
    # all images' box rows (5K each) + onesb row, single partition row
    d_boxall = nc.dram_tensor("boxall", [1, I * 5 * K + 128], BF16,
                              kind="ExternalInput")
    # gather stationaries for all images, p-major
    d_qall = nc.dram_tensor("qall", [128, I * 8 * NQ], BF16,
                            kind="ExternalInput")
    d_lblall = nc.dram_tensor("lblall", [K, I], F32, kind="ExternalInput")
    # 1/(parea + barea) per image, p-major bf16 (host-precomputed)
    d_rA = nc.dram_tensor("rA", [I, 128, T * K], BF16, kind="ExternalInput")
    d_ident = nc.dram_tensor("ident", [128, 128], BF16, kind="ExternalInput")
    d_ones_r = nc.dram_tensor("ones_r", [1, 128], F32, kind="ExternalInput")
    d_iota81 = nc.dram_tensor("iota81", [K, C], F32, kind="ExternalInput")
    # out: per-partition partials; host does the final sums/max.
    # cols: [0:16] scadd (i,4: F, cnt_hi, lps, box), [16:20] ufa (rows 0:K),
    # [20:24] bm, [24:28] np (replicated)
    d_out = nc.dram_tensor("out", [128, 28], F32, kind="ExternalOutput")

    from contextlib import ExitStack

    with tile.TileContext(nc) as tc, ExitStack() as es:
        cpool = es.enter_context(tc.tile_pool(name="consts", bufs=1))
        spool = es.enter_context(tc.tile_pool(name="scores", bufs=1))
        prepool = es.enter_context(tc.tile_pool(name="prein", bufs=1))
        wpool = es.enter_context(tc.tile_pool(name="work", bufs=1))
        epool = es.enter_context(tc.tile_pool(name="exp", bufs=2))
        bpool = es.enter_context(tc.tile_pool(name="batched", bufs=1))
        pp_t = es.enter_context(tc.tile_pool(name="ps_t", bufs=1, space="PSUM"))
        pp_sel = es.enter_context(tc.tile_pool(name="ps_sel", bufs=1, space="PSUM"))
        pp_u = es.enter_context(tc.tile_pool(name="ps_u", bufs=1, space="PSUM"))
        pp_r = es.enter_context(tc.tile_pool(name="ps_r", bufs=2, space="PSUM"))
        rpool = es.enter_context(tc.tile_pool(name="redsb", bufs=4))

        # ------- constants + all per-image loads, DMA-issue spread over
        # ------- SP/ACT queues (engine-synchronous issue serializes)
        cbf = cpool.tile([128, B_TOT], BF16, tag="cbf")
        ioj = cpool.tile([128, 16, TG], BF16, tag="ioj")
        cf32 = cpool.tile([128, C_TOT], F32, tag="cf32")
        ident = cpool.tile([128, 128], BF16, tag="ident")
        ones_r = cpool.tile([1, 128], F32, tag="ones_r")
        iota81 = cpool.tile([K, C], F32, tag="iota81")

        sres_all = spool.tile([128, I, T, C], F8, tag="sres")
        rA_l = [prepool.tile([128, T, K], BF16, tag="rA", bufs=2,
                             name=f"rA{i}") for i in range(I)]
        boxall = cpool.tile([1, I * 5 * K + 128], BF16, tag="boxall")
        qall = cpool.tile([128, I, 8 * NQ], BF16, tag="qall")
        lblall = cpool.tile([K, I], F32, tag="lblall")
        l4all = cpool.tile([128, I, T, 4], F32, tag="l4all")
        boxf_l = [boxall[:, i * 5 * K:(i + 1) * 5 * K]
                  .rearrange("p (a k) -> p a k", k=K) for i in range(I)]
        onesb = boxall[:, I * 5 * K:]
        qblk_l = [qall[:, i, :] for i in range(I)]
        lbl_l = [lblall[:, i:i + 1] for i in range(I)]

        # ACT queue: box/onesb row first (bb broadcasts unblock), then rA0
        nc.scalar.dma_start(out=boxall[:], in_=d_boxall[:, :])
        nc.scalar.dma_start(out=rA_l[0][:].rearrange("p t k -> p (t k)"),
                            in_=d_rA[0, :, :])
        # SP queue: jaccard consts first, then everything else by need time
        nc.sync.dma_start(out=cbf[:], in_=d_cbf[:, :])
        nc.sync.dma_start(out=cf32[:], in_=d_cf32[:, :])
        nc.sync.dma_start(out=ones_r[:], in_=d_ones_r[:, :])
        nc.sync.dma_start(out=ident[:], in_=d_ident[:, :])
        nc.sync.dma_start(out=rA_l[1][:].rearrange("p t k -> p (t k)"),
                          in_=d_rA[1, :, :])
        nc.sync.dma_start(out=rA_l[2][:].rearrange("p t k -> p (t k)"),
                          in_=d_rA[2, :, :])
        nc.sync.dma_start(out=rA_l[3][:].rearrange("p t k -> p (t k)"),
                          in_=d_rA[3, :, :])
        nc.sync.dma_start(out=qall[:].rearrange("p i q -> p (i q)"),
                          in_=d_qall[:, :])
        nc.sync.dma_start(out=ioj[:].rearrange("p j t -> p (j t)"),
                          in_=d_ioj[:, :])
        nc.sync.dma_start(
            out=sres_all[:, 0].rearrange("p t c -> p (t c)"),
            in_=d_scores[0, :, :])
        nc.sync.dma_start(
            out=sres_all[:, 1].rearrange("p t c -> p (t c)"),
            in_=d_scores[1, :, :])
        nc.sync.dma_start(
            out=sres_all[:, 2].rearrange("p t c -> p (t c)"),
            in_=d_scores[2, :, :])
        nc.sync.dma_start(
            out=sres_all[:, 3].rearrange("p t c -> p (t c)"),
            in_=d_scores[3, :, :])
        nc.sync.dma_start(out=lblall[:], in_=d_lblall[:, :])
        nc.sync.dma_start(out=l4all[:].rearrange("p i t d -> p (i t d)"),
                          in_=d_locs[:, :])
        nc.sync.dma_start(out=iota81[:], in_=d_iota81[:, :])

        # warm the ACT function table immediately (the implicit
        # ACT_TABLE_LOAD otherwise blocks the first bb copy ~5us later)
        dwarm = rpool.tile([1, 1], F32, tag="dwarm")
        nc.gpsimd.memset(dwarm[:], 0.0)
        nc.scalar.activation(dwarm[:], dwarm[:], ACTF.Relu)

        def prow(off):  # bf16 pre-expanded prior row view [128, T, K]
            return cbf[:, off:off + T * K].rearrange("p (t k) -> p t k", k=K)

        pxe = {nm: prow(off) for nm, off in
               [("px1", B_PX1), ("py1", B_PY1), ("px2", B_PX2),
                ("py2", B_PY2)]}
        iotaJ = ioj[:]
        iv4 = cf32[:, C_IV4:C_PC4].rearrange("p (t d) -> p t d", d=4)
        pc4 = cf32[:, C_PC4:C_ONE].rearrange("p (t d) -> p t d", d=4)
        ones_p = cf32[:, C_ONE:C_IDF]
        identf = cf32[:, C_IDF:C_O128]
        ones128 = cf32[:, C_O128:C_TOT]

        def rowsum(dst_row_ap, src_ap, n):
            """[P, n] f32 -> [1, n] partition sum written to dst_row_ap."""
            ps = pp_r.tile([128, 128], F32, tag="red_bc")
            nc.tensor.matmul(ps[0:1, :n], lhsT=ones_p[:src_ap.shape[0], :],
                             rhs=src_ap, start=True, stop=True)
            nc.scalar.copy(dst_row_ap, ps[0:1, :n])

        def bcast_row(dst_ap, row_ap, n):
            """[1, n] f32 -> [128, n] replicated (dst may be bf16)."""
            ps = pp_r.tile([128, 128], F32, tag="red_bc")
            nc.tensor.matmul(ps[:, :n], lhsT=ones_r[:], rhs=row_ap,
                             start=True, stop=True)
            nc.scalar.copy(dst_ap, ps[:, :n])

        def allreduce_sum(dst_ap, src_ap, n):
            ps = pp_r.tile([128, 128], F32, tag="red_bc")
            nc.tensor.matmul(ps[:, :n], lhsT=ones128, rhs=src_ap,
                             start=True, stop=True)
            nc.scalar.copy(dst_ap, ps[:, :n])

        def maxreduce_row(dst_row_ap, src_ap, n):
            """[128, n] f32 -> [1, n] partition max written to dst_row_ap."""
            ps = pp_r.tile([128, 128], F32, tag="red_bc")
            nc.tensor.transpose(ps[:n, :], src_ap, identf)
            tsb = rpool.tile([128, 128], F32, tag="red_tsb")
            nc.scalar.copy(tsb[:n, :], ps[:n, :])
            mx = rpool.tile([128, 1], F32, tag="red_mx")
            nc.vector.tensor_reduce(out=mx[:n, :], in_=tsb[:n, :],
                                    axis=AX.X, op=OP.max)
            ps2 = pp_r.tile([128, 128], F32, tag="red_bc")
            nc.tensor.transpose(ps2[0:1, :n], mx[:n, :], identf[:n, :n])
            nc.scalar.copy(dst_row_ap, ps2[0:1, :n])

        # batched buffers [128, I, ...]
        lse4 = bpool.tile([128, I, T], F32, tag="lse4")
        cen4 = bpool.tile([128, I, T], BF16, tag="cen4")
        pos4 = bpool.tile([128, I, T], F32, tag="pos4")
        out_sb = bpool.tile([128, 28], F32, tag="out_sb")
        np4 = out_sb[:, 24:28]
        npt4 = bpool.tile([128, I], F32, tag="npt4")
        k34 = bpool.tile([128, I], F32, tag="k34")
        cnt_all = bpool.tile([128, I, 16], F32, tag="cnt_all")
        cntr = bpool.tile([128, I, 16], F32, tag="cntr")
        lo4 = bpool.tile([128, I], F32, tag="lo4")
        hi4 = bpool.tile([128, I], F32, tag="hi4")
        scadd = out_sb[:, 0:16].rearrange("p (i s) -> p i s", s=4)
        ufa4 = out_sb[0:K, 16:20]
        bm4 = out_sb[:, 20:24]
        # counting grid scratch (level-1 batched across images; level-2
        # reuses per-image slices of the same tiles)
        cen16 = bpool.tile([128, I, TG], BF16, tag="cen16")
        mask1 = bpool.tile([128, I, 16, TG], BF16, tag="mask1")
        m40a = bpool.tile([128, I, 16, 40], BF16, tag="m40a")
        m20a = bpool.tile([128, I, 16, 20], BF16, tag="m20a")
        m10a = bpool.tile([128, I, 16, 10], BF16, tag="m10a")
        m5a = bpool.tile([128, I, 16, 5], BF16, tag="m5a")
        ge4 = bpool.tile([128, I, 16], F32, tag="ge4")
        mc4 = bpool.tile([128, I], F32, tag="mc4")
        fsc4 = bpool.tile([128, I, T], BF16, tag="fsc4")
        lpsb = bpool.tile([128, I, T], F32, tag="lpsb")
        ce0 = bpool.tile([128, I, T], F32, tag="ce0")

        # grid pads: cen16 pad cols stay 0 (grid pad is 999 -> mask 0)
        nc.gpsimd.memset(cen16[:, :, T:], 0.0)

        # ---------------- per-image box rows via PE broadcast -------------
        bb_l = []
        for i in range(I):
            bbt = prepool.tile([128, 5, K], BF16, tag="bb", bufs=4,
                               name=f"bb{i}")
            ps_bb = pp_r.tile([128, 128], F32, tag="red_bc",
                              name=f"psbb{i}")
            nc.tensor.matmul(ps_bb[:, :5 * K], lhsT=onesb,
                             rhs=boxf_l[i].rearrange("p a k -> p (a k)"),
                             start=True, stop=True)
            nc.scalar.copy(bbt[:].rearrange("p a k -> p (a k)"),
                           ps_bb[:, :5 * K])
            bb_l.append(bbt)

        def emit_J(i):
            """Jaccard + per-box max for image i (DVE-heavy, few stalls)."""
            bb = bb_l[i]

            def bcast_b(row):  # [128, K] box row -> [128, T, K] AP (packed k)
                return bb[:, row, :][:, None, :].broadcast_to([128, T, K])

            # ---------------- jaccard (linear ratio domain, bf16) ---------
            ovp80 = wpool.tile([128, T8, K], BF16, tag="ovp80", bufs=2)
            if i < 2:
                nc.gpsimd.memset(ovp80[:, T:, :], -1.0)
            ov = ovp80[:, :T, :]
            ltxy = wpool.tile([128, 2, T, K], BF16, tag="ltxy")
            w0h0 = wpool.tile([128, 2, T, K], BF16, tag="w0h0")
            wrhr = wpool.tile([128, 2, T, K], BF16, tag="wrhr", bufs=2)
            inter = wpool.tile([128, T, K], BF16, tag="inter")

            # x and y stacked on a free dim: max/min/sub are one op each
            px12 = cbf[:, B_PX1:B_PX1 + 2 * T * K].rearrange(
                "p (r t k) -> p r t k", r=2, k=K)
            px34 = cbf[:, B_PX2:B_PX2 + 2 * T * K].rearrange(
                "p (r t k) -> p r t k", r=2, k=K)
            b01 = bb[:, 0:2, :][:, :, None, :].broadcast_to([128, 2, T, K])
            b23 = bb[:, 2:4, :][:, :, None, :].broadcast_to([128, 2, T, K])
            nc.vector.tensor_tensor(out=ltxy[:], in0=px12, in1=b01,
                                    op=OP.max)
            nc.vector.tensor_tensor(out=w0h0[:], in0=px34, in1=b23,
                                    op=OP.min)
            nc.vector.tensor_sub(wrhr[:], w0h0[:], ltxy[:])
            nc.scalar.activation(wrhr[:], wrhr[:], ACTF.Relu)
            nc.vector.tensor_mul(inter[:], wrhr[:, 0], wrhr[:, 1])
            nc.vector.tensor_mul(ov, inter[:], rA_l[i][:])

            # per-box max over priors: dense max tree (80 = 2*2*2*2*5),
            # then the cross-partition max (PE transpose round trip)
            tm1 = wpool.tile([128, 40, K], BF16, tag="tm1")
            nc.vector.tensor_tensor(out=tm1[:], in0=ovp80[:, :40, :],
                                    in1=ovp80[:, 40:, :], op=OP.max)
            tm2 = wpool.tile([128, 20, K], BF16, tag="tm2")
            nc.vector.tensor_tensor(out=tm2[:], in0=tm1[:, :20, :],
                                    in1=tm1[:, 20:, :], op=OP.max)
            tm3 = wpool.tile([128, 10, K], BF16, tag="tm3")
            nc.vector.tensor_tensor(out=tm3[:], in0=tm2[:, :10, :],
                                    in1=tm2[:, 10:, :], op=OP.max)
            tm4 = wpool.tile([128, 5, K], BF16, tag="tm4")
            nc.vector.tensor_tensor(out=tm4[:], in0=tm3[:, :5, :],
                                    in1=tm3[:, 5:, :], op=OP.max)
            m16 = wpool.tile([128, K], F32, tag="m16", bufs=2)
            nc.vector.tensor_reduce(
                out=m16[:], in_=tm4[:].rearrange("p t k -> p k t"),
                axis=AX.X, op=OP.max)
            m16row = wpool.tile([1, K], F32, tag="m16row", bufs=2)
            maxreduce_row(m16row[:], m16[:], K)
            return {"ov": ov, "m16row": m16row}

        def emit_F(i, st):
            """Forcing + one-hot + gather + L1 + U for image i."""
            ov = st["ov"]
            l4 = l4all[:, i]
            qblk = qblk_l[i]
            lbl16 = lbl_l[i]
            sres = sres_all[:, i]

            m16rb = wpool.tile([128, K], BF16, tag="m16rb", bufs=2)
            bcast_row(m16rb[:], st["m16row"][:], K)
            fmask = wpool.tile([128, T, K], BF16, tag="fmask")
            nc.vector.tensor_tensor(
                out=fmask[:], in0=ov,
                in1=m16rb[:][:, None, :].broadcast_to([128, T, K]),
                op=OP.is_equal)
            # uniform sentinel 101 (multi-forced priors go multi-hot; rare
            # and bounded): fm2 = fmask*102 - 1 in {-1, 101}
            ovf = wpool.tile([128, T, K], BF16, tag="ovf", bufs=2)
            fm2 = wpool.tile([128, T, K], BF16, tag="fm2")
            nc.vector.tensor_scalar(out=fm2[:], in0=fmask[:],
                                    scalar1=102.0, scalar2=-1.0,
                                    op0=OP.mult, op1=OP.add)
            nc.vector.tensor_tensor(out=ovf[:], in0=fm2[:], in1=ov,
                                    op=OP.max)
            # per-prior max over k: dense tree on the packed innermost dim
            # per-prior max over k: dense tree on the packed innermost dim
            ms1 = wpool.tile([128, T, 8], BF16, tag="ms1")
            nc.vector.tensor_tensor(out=ms1[:], in0=ovf[:, :, 0:8],
                                    in1=ovf[:, :, 8:16], op=OP.max)
            ms2 = wpool.tile([128, T, 4], BF16, tag="ms2")
            nc.vector.tensor_tensor(out=ms2[:], in0=ms1[:, :, 0:4],
                                    in1=ms1[:, :, 4:8], op=OP.max)
            ms3 = wpool.tile([128, T, 2], BF16, tag="ms3")
            nc.vector.tensor_tensor(out=ms3[:], in0=ms2[:, :, 0:2],
                                    in1=ms2[:, :, 2:4], op=OP.max)
            pm = wpool.tile([128, T], BF16, tag="pm")
            nc.vector.tensor_tensor(out=pm[:], in0=ms3[:, :, 0],
                                    in1=ms3[:, :, 1], op=OP.max)
            # pmz = pm where positive else pm+1 (matches nothing): fuses the
            # one-hot and the pos mask into a single is_eq
            pmz = wpool.tile([128, T], BF16, tag="pmz")
            nc.vector.scalar_tensor_tensor(
                out=pmz[:], in0=pm[:], scalar=THR, in1=pm[:],
                op0=OP.is_lt, op1=OP.add)
            # expand pmz across k on ACT so the is_eq runs packed on DVE
            pmze = wpool.tile([128, T, K], BF16, tag="pmze", bufs=2)
            nc.scalar.copy(
                pmze[:], pmz[:][:, :, None].broadcast_to([128, T, K]))
            wm72 = wpool.tile([128, T2 * K], BF16, tag="wm72", bufs=2)
            if i < 2:
                nc.gpsimd.memset(wm72[:, T * K:], 0.0)
            wmat = wm72[:, :T * K].rearrange("p (t k) -> p t k", k=K)
            nc.vector.tensor_tensor(out=wmat, in0=ovf[:], in1=pmze[:],
                                    op=OP.is_equal)
            nc.vector.tensor_scalar(out=pos4[:, i, :], in0=pm[:],
                                    scalar1=THR, scalar2=None,
                                    op0=OP.is_ge, op1=OP.add,
                                    accum_out=npt4[:, i:i + 1])

            # ---------------- box gather via PE ----------------
            ohT_ps = pp_t.tile([128, NB, 128], BF16, tag="ohT")
            for b in range(NB):
                nc.tensor.transpose(
                    ohT_ps[:, b, :],
                    wm72[:, b * 128:(b + 1) * 128],
                    ident[:])
            ohT_sb = wpool.tile([128, NB * 128], BF16, tag="ohT_sb", bufs=2)
            nc.scalar.copy(ohT_sb[:], ohT_ps[:].rearrange("p b n -> p (b n)"))

            sel_ps = pp_sel.tile([8 * NQ, NB, 128], F32, tag="sel")
            for b in range(NB):
                nc.tensor.matmul(sel_ps[:, b, :], lhsT=qblk,
                                 rhs=ohT_sb[:, b * 128:(b + 1) * 128],
                                 start=True, stop=True)
            sel_sb = wpool.tile([8 * NQ, NB * 128], BF16, tag="sel_sb",
                                bufs=2)
            nc.scalar.copy(sel_sb[:], sel_ps[:].rearrange("p b n -> p (b n)"))
            bk_ps = pp_t.tile([128, NB, 8 * NQ], BF16, tag="ohT")
            for b in range(NB):
                nc.tensor.transpose(
                    bk_ps[:, b, :],
                    sel_sb[:, b * 128:(b + 1) * 128],
                    ident[:8 * NQ, :8 * NQ])
            selq = wpool.tile([128, NB * 8 * NQ], BF16, tag="selq", bufs=2)
            nc.scalar.copy(selq[:], bk_ps[:].rearrange("p b n -> p (b n)"))
            # selq[p, (blk*40 + tb*5 + q)] = sel_q at t = blk*8+tb
            sel4 = selq[:].rearrange("p (t q) -> p t q", q=NQ)[:, :, 0:4]

            # ---------------- box L1 (Pool chain + ACT abs-accum) ---------
            lp4 = wpool.tile([128, T, 4], F32, tag="lp4")
            nc.gpsimd.tensor_add(lp4[:], l4, pc4[:, :T, :])
            tb1 = wpool.tile([128, T, 4], F32, tag="tb1")
            nc.gpsimd.tensor_mul(tb1[:], sel4[:, :T, :], iv4[:, :T, :])
            nc.gpsimd.tensor_sub(tb1[:], lp4[:], tb1[:])
            nc.vector.tensor_tensor(
                out=tb1[:], in0=tb1[:],
                in1=pos4[:, i, :][:, :, None].broadcast_to([128, T, 4]),
                op=OP.mult)
            nc.scalar.activation(tb1[:], tb1[:], ACTF.Abs,
                                 accum_out=scadd[:, i, 3:4])

            # ---------------- U matrix (score at label) ----------------
            u_ps = pp_u.tile([K, C], F32, tag="u")
            for t_ in range(T):
                nc.tensor.matmul(u_ps[:], lhsT=wmat[:, t_, :],
                                 rhs=sres[:, t_, :],
                                 start=(t_ == 0), stop=(t_ == T - 1))
            u_sb = wpool.tile([K, C], F32, tag="u_sb", bufs=4)
            nc.scalar.copy(u_sb[:], u_ps[:])
            u_sb_l.append(u_sb)

        # sequential per image: higher cross-engine concurrency (software
        # pipelining J/F) measured ~20% slower per-op from SBUF contention
        u_sb_l = []
        for i in range(I):
            emit_F(i, emit_J(i))

        # ------- CE: exp (ACT) + class sums (bf16 DVE add tree),
        # ------- batched across all I images per chunk -----------------
        for ch in range(NCH):
            et = epool.tile([128, I, CT, C], BF16, tag="et", bufs=3)
            nc.scalar.activation(
                et[:], sres_all[:, :, ch * CT:(ch + 1) * CT, :], ACTF.Exp)
            e3 = et[:].rearrange("p i t c -> p (i t) c")
            t40 = epool.tile([128, I * CT, 40], BF16, tag="t40")
            t20 = epool.tile([128, I * CT, 20], BF16, tag="t20")
            t10 = epool.tile([128, I * CT, 10], BF16, tag="t10")
            t5 = epool.tile([128, I * CT, 5], BF16, tag="t5")
            secc = epool.tile([128, I, CT], F32, tag="secc")
            with nc.allow_low_precision("bf16 class sums"):
                nc.vector.tensor_add(t40[:], e3[:, :, 0:40], e3[:, :, 40:80])
                nc.vector.tensor_add(t20[:], t40[:, :, 0:20],
                                     t40[:, :, 20:40])
                nc.vector.tensor_add(t10[:], t20[:, :, 0:10],
                                     t20[:, :, 10:20])
                nc.vector.tensor_add(t5[:], t10[:, :, 0:5], t10[:, :, 5:10])
            nc.vector.tensor_reduce(
                out=secc[:].rearrange("p i t -> p (i t)"), in_=t5[:],
                axis=AX.X, op=OP.add)
            nc.vector.tensor_add(secc[:], secc[:], et[:, :, :, 80])
            nc.scalar.activation(
                lse4[:, :, ch * CT:(ch + 1) * CT], secc[:], ACTF.Ln)

        # ce0/cen/lps batched
        nc.vector.tensor_sub(ce0[:], lse4[:], sres_all[:, :, :, 0])
        nc.vector.scalar_tensor_tensor(
            out=cen4[:], in0=pos4[:], scalar=0.5,
            in1=ce0[:], op0=OP.is_lt, op1=OP.mult)

        # n_pos allreduce + k3, batched
        allreduce_sum(np4[:], npt4[:], I)
        nc.vector.tensor_scalar(out=k34[:], in0=np4[:], scalar1=3.0,
                                scalar2=None, op0=OP.mult)

        # ---- counting level 1, batched: 16 counts per image via one
        # ---- is_gt against the j/16 grid (cen/16 is exact in bf16)
        nc.vector.tensor_scalar(out=cen16[:, :, :T], in0=cen4[:],
                                scalar1=1.0 / 16, scalar2=None, op0=OP.mult)
        nc.vector.tensor_tensor(
            out=mask1[:],
            in0=cen16[:, :, None, :].broadcast_to([128, I, 16, TG]),
            in1=iotaJ[:, None, :, :].broadcast_to([128, I, 16, TG]),
            op=OP.is_gt)
        with nc.allow_low_precision("bf16 count sums"):
            nc.vector.tensor_add(m40a[:], mask1[:, :, :, 0:40],
                                 mask1[:, :, :, 40:80])
            nc.vector.tensor_add(m20a[:], m40a[:, :, :, 0:20],
                                 m40a[:, :, :, 20:40])
            nc.vector.tensor_add(m10a[:], m20a[:, :, :, 0:10],
                                 m20a[:, :, :, 10:20])
            nc.vector.tensor_add(m5a[:], m10a[:, :, :, 0:5],
                                 m10a[:, :, :, 5:10])
        nc.vector.tensor_reduce(
            out=cnt_all[:].rearrange("p i j -> p (i j)"), in_=m5a[:],
            axis=AX.X, op=OP.add)
        allreduce_sum(cntr[:].rearrange("p i j -> p (i j)"),
                      cnt_all[:].rearrange("p i j -> p (i j)"), I * 16)

        # fill the count-allreduce round trip: lps partial + U-label pick
        nc.vector.tensor_mul(lpsb[:], pos4[:], lse4[:])
        for i in range(I):
            ufx = wpool.tile([K, C], F32, tag="ufx")
            nc.vector.scalar_tensor_tensor(
                out=ufx[:], in0=iota81[:], scalar=lbl_l[i], in1=u_sb_l[i][:],
                op0=OP.is_equal, op1=OP.mult, accum_out=ufa4[:, i:i + 1])

        # ---------------- mining: lo per image, then level-2 batched -----
        for i in range(I):
            # lo = (#edges with count >= k) - 1   (edges j = 0..15)
            nc.vector.tensor_scalar(out=ge4[:, i, :], in0=cntr[:, i, :],
                                    scalar1=k34[:, i:i + 1], scalar2=None,
                                    op0=OP.is_ge, op1=OP.add,
                                    accum_out=lo4[:, i:i + 1])
        nc.vector.tensor_scalar(out=lo4[:], in0=lo4[:],
                                scalar1=-1.0, scalar2=None, op0=OP.add)
        # level 2: thresholds lo + j/16 via (cen - lo) > j/16, all images
        for i in range(I):
            nc.vector.tensor_scalar(out=cen16[:, i, :T],
                                    in0=cen4[:, i, :],
                                    scalar1=lo4[:, i:i + 1],
                                    scalar2=None, op0=OP.subtract)
        nc.vector.tensor_tensor(
            out=mask1[:],
            in0=cen16[:, :, None, :].broadcast_to([128, I, 16, TG]),
            in1=iotaJ[:, None, :, :].broadcast_to([128, I, 16, TG]),
            op=OP.is_gt)
        with nc.allow_low_precision("bf16 count sums"):
            nc.vector.tensor_add(m40a[:], mask1[:, :, :, 0:40],
                                 mask1[:, :, :, 40:80])
            nc.vector.tensor_add(m20a[:], m40a[:, :, :, 0:20],
                                 m40a[:, :, :, 20:40])
            nc.vector.tensor_add(m10a[:], m20a[:, :, :, 0:10],
                                 m20a[:, :, :, 10:20])
            nc.vector.tensor_add(m5a[:], m10a[:, :, :, 0:5],
                                 m10a[:, :, :, 5:10])
        nc.vector.tensor_reduce(
            out=cnt_all[:].rearrange("p i j -> p (i j)"), in_=m5a[:],
            axis=AX.X, op=OP.add)
        allreduce_sum(cntr[:].rearrange("p i j -> p (i j)"),
                      cnt_all[:].rearrange("p i j -> p (i j)"), I * 16)
        nc.vector.tensor_reduce(out=scadd[:, :, 2], in_=lpsb[:],
                                axis=AX.X, op=OP.add)
        for i in range(I):
            nc.vector.tensor_scalar(out=ge4[:, i, :], in0=cntr[:, i, :],
                                    scalar1=k34[:, i:i + 1], scalar2=None,
                                    op0=OP.is_ge, op1=OP.add,
                                    accum_out=mc4[:, i:i + 1])
        nc.vector.tensor_scalar(out=mc4[:], in0=mc4[:], scalar1=1.0 / 16,
                                scalar2=None, op0=OP.mult)
        nc.vector.tensor_add(hi4[:], mc4[:], lo4[:])
        # F(hi), count(hi), boundary max per image (independent chains)
        for i in range(I):
            nc.vector.scalar_tensor_tensor(
                out=fsc4[:, i, :], in0=cen4[:, i, :], scalar=hi4[:, i:i + 1],
                in1=cen4[:, i, :], op0=OP.is_gt, op1=OP.mult,
                accum_out=scadd[:, i, 0:1])
            nc.vector.tensor_scalar(out=fsc4[:, i, :], in0=cen4[:, i, :],
                                    scalar1=hi4[:, i:i + 1], scalar2=None,
                                    op0=OP.is_gt, op1=OP.add,
                                    accum_out=scadd[:, i, 1:2])
            nc.vector.scalar_tensor_tensor(
                out=fsc4[:, i, :], in0=cen4[:, i, :], scalar=hi4[:, i:i + 1],
                in1=cen4[:, i, :], op0=OP.is_le, op1=OP.mult)
            nc.vector.tensor_reduce(out=bm4[:, i:i + 1], in_=fsc4[:, i, :],
                                    axis=AX.X, op=OP.max)

        nc.sync.dma_start(out=d_out[:, :], in_=out_sb[:])

    if fixup:
        _fixup_module(nc)
    return nc


def prepare_inputs(predicted_locs, predicted_scores, boxes, labels,
                   priors_centers):
    """Shard + marshal the full inputs into 8 per-core in_maps (p-major)."""
    predicted_locs = np.asarray(predicted_locs, np.float32)
    predicted_scores = np.asarray(predicted_scores, np.float32)
    boxes = np.asarray(boxes, np.float32)
    labels_f = np.asarray(labels).astype(np.float32)
    priors = np.asarray(priors_centers, np.float32)

    npad = PP - P
    # scores: pad rows have class0=0, others -50 -> lse=0, S0=0, ce0=0 exactly
    pad_scores = np.full((B, npad, C), -50.0, np.float32)
    pad_scores[:, :, 0] = 0.0
    scores_p = np.concatenate([predicted_scores, pad_scores], axis=1)
    # p-major: [B, 128, T*C]
    scores_pm = np.ascontiguousarray(
        scores_p.reshape(B, T, 128, C).transpose(0, 2, 1, 3)
    ).reshape(B, 128, T * C)
    scores_bf = scores_pm.astype(ml_dtypes.float8_e4m3)
    locs_p = np.concatenate(
        [predicted_locs, np.zeros((B, npad, 4), np.float32)], axis=1)
    locs_pm = np.ascontiguousarray(
        locs_p.reshape(B, T, 128, 4).transpose(0, 2, 1, 3)
    ).reshape(B, 128, T * 4)

    # prior rows pre-expanded across K (p-major, bf16)
    pad_pri = np.tile(np.array([-100.0, -100.0, 1.0, 1.0], np.float32),
                      (npad, 1))
    pri = np.concatenate([priors, pad_pri], axis=0)
    pcx, pcy, pw, ph = pri[:, 0], pri[:, 1], pri[:, 2], pri[:, 3]

    def pm_grid(v):  # [PP] -> [128, T]
        return np.ascontiguousarray(v.astype(np.float32).reshape(T, 128).T)

    def pexp(v):  # [PP] -> [128, T*K] expanded across K
        return np.repeat(pm_grid(v)[:, :, None], K, axis=2).reshape(128, T * K)

    # counting threshold grid [16, TG]: j/16 for t<T, 999 on pads
    ioj = np.full((16, TG), 999.0, np.float32)
    ioj[:, :T] = (np.arange(16, dtype=np.float32) / 16.0)[:, None]
    ioj_b = np.tile(ioj.reshape(1, 16 * TG), (128, 1))
    cbf = _to_bf16(np.concatenate(
        [pexp(pcx - pw / 2), pexp(pcy - ph / 2), pexp(pcx + pw / 2),
         pexp(pcy + ph / 2)], axis=1))
    assert cbf.shape[1] == B_TOT
    ioj_bf = _to_bf16(ioj_b)

    # iv4/pc4 [128, T2, 4] host-assembled (d = x, y, w, h; tail zero)
    iv4 = np.zeros((128, T2, 4), np.float32)
    pc4 = np.zeros((128, T2, 4), np.float32)
    iv4[:, :T, 0] = pm_grid(10.0 / pw)
    iv4[:, :T, 1] = pm_grid(10.0 / ph)
    iv4[:, :T, 2] = 1.0
    iv4[:, :T, 3] = 1.0
    pc4[:, :T, 0] = pm_grid(pcx * (10.0 / pw))
    pc4[:, :T, 1] = pm_grid(pcy * (10.0 / ph))
    pc4[:, :T, 2] = pm_grid(5.0 * np.log(pw))
    pc4[:, :T, 3] = pm_grid(5.0 * np.log(ph))

    ones_p = np.ones((128, 1), np.float32)
    identf = np.eye(128, dtype=np.float32)
    cf32 = np.concatenate(
        [iv4.reshape(128, T2 * 4), pc4.reshape(128, T2 * 4),
         ones_p, identf, np.ones((128, 128), np.float32)],
        axis=1).astype(np.float32)
    assert cf32.shape[1] == C_TOT

    bx1, by1, bx2, by2 = (boxes[:, :, d] for d in range(4))
    barea = (bx2 - bx1) * (by2 - by1)
    boxf = np.stack([bx1, by1, bx2, by2, barea], axis=1)      # [B, 5, K]
    # rA[b, p, t, k] = 1/(parea[p,t] + barea[b,k]) (p-major)
    parea_pm = pm_grid(pw * ph)                               # [128, T]
    rA = _to_bf16(1.0 / (
        parea_pm[None, :, :, None] + barea[:, None, None, :].astype(np.float64)
    ).reshape(B, 128, T * K))
    q5 = np.stack([
        (bx1 + bx2) / 2, (by1 + by2) / 2,
        5.0 * np.log(bx2 - bx1), 5.0 * np.log(by2 - by1),
        np.zeros_like(bx1),
    ], axis=2).astype(np.float32)                           # [B, K, 5]
    qblk_f = np.zeros((B, 128, 8 * NQ), np.float32)
    for tb in range(8):
        qblk_f[:, tb * K:(tb + 1) * K, tb * NQ:(tb + 1) * NQ] = q5

    ident = _to_bf16(np.eye(128, dtype=np.float32))
    ones_r = np.ones((1, 128), np.float32)
    iota81 = np.tile(np.arange(C, dtype=np.float32), (K, 1))

    in_maps = []
    for c in range(NCORES):
        sl = slice(c * I, (c + 1) * I)
        boxall = _to_bf16(np.concatenate(
            [boxf[sl].reshape(-1), np.ones(128, np.float32)])[None, :])
        qall = _to_bf16(np.ascontiguousarray(
            qblk_f[sl].transpose(1, 0, 2)).reshape(128, I * 8 * NQ))
        lblall = np.ascontiguousarray(labels_f[sl].T)         # [K, I]
        locs2 = np.ascontiguousarray(
            locs_pm[sl].transpose(1, 0, 2)).reshape(128, I * T * 4)
        in_maps.append({
            "scores": scores_bf[sl],
            "locs": locs2,
            "cbf": cbf,
            "ioj": ioj_bf,
            "cf32": cf32,
            "boxall": boxall,
            "qall": qall,
            "lblall": lblall,
            "rA": rA[sl],
            "ident": ident,
            "ones_r": ones_r,
            "iota81": iota81,
        })
    return in_maps


def combine_outputs(outs):
    """outs: list of 8 per-core [128, 28] partials -> scalar loss."""
    n_pos_total = 0.0
    box_sum = 0.0
    class_sum = 0.0
    for o in outs:
        o = np.asarray(o, np.float64)
        sc = o[:, :16].reshape(128, I, 4).sum(axis=0)   # [I, 4]
        uf = o[:K, 16:20].sum(axis=0)                   # [I]
        bm = o[:, 20:24].max(axis=0)                    # [I]
        npi = o[0, 24:28]                               # [I]
        f_sum, cnt_hi, lps, box = sc[:, 0], sc[:, 1], sc[:, 2], sc[:, 3]
        r = (3.0 * npi - cnt_hi) * bm + f_sum
        cep = lps - uf
        n_pos_total += npi.sum()
        box_sum += box.sum()
        class_sum += (cep + r).sum()
    loss = class_sum / n_pos_total + box_sum / (n_pos_total * 4.0)
    return np.float32(loss)


_NC_CACHE = {}


def kernel(predicted_locs, predicted_scores, boxes, labels, priors_centers):
    if "nc" not in _NC_CACHE:
        _NC_CACHE["nc"] = build_nc()
    nc = _NC_CACHE["nc"]
    in_maps = prepare_inputs(predicted_locs, predicted_scores, boxes, labels,
                             priors_centers)
    res = run_bass_kernel_spmd(nc, in_maps, list(range(NCORES)))
    outs = [res.results[c]["out"] for c in range(NCORES)]
    return combine_outputs(outs)


if __name__ == "__main__":
    import reference as R

    inputs = {k: np.asarray(v) for k, v in R.setup_inputs().items()}
    print("loss =", kernel(**inputs))


# revision 85
# speedup vs baseline: 1.1280x; 1.0113x over previous
"""Trainium2 Bass kernel for SSD MultiBox loss (nn_ModelLoss_5970004541458).

Strategy: data-parallel over batch (32 images -> 8 cores x 4 images).
Per core, everything over the prior dim (P=8732, padded to 8960 = 70*128)
runs on-device:
  - jaccard matching (16 boxes x 8960 priors) in bf16 in the LINEAR
    ratio domain: ov = inter * (1/(parea+barea)) (recipA host-precomputed)
    is a monotone bijection of IoU, so max/argmax/threshold semantics
    match IoU with thr 1/3.  Prior rows are pre-expanded across K on the
    host so DVE tensor ops have packed 2-byte operands.
    Forced-assignment sentinels: ovf = max(fmask*102 - 1, ov) -> forced
    elements read 101 (above any real ratio <= 0.5), others unchanged.
  - per-prior one-hot box gather via PE transpose + block-diag matmul (bf16)
  - CE: exp on ACT batched across all 4 images per chunk; class sums via
    a bf16 pairwise add tree at DVE 2x mode
  - hard-negative mining via a 2-level 16-way counting grid with bounded-
    error boundary correction; each level's 16 counts come from ONE
    is_gt against a host-provided threshold grid + a bf16 add tree
    (instead of 16 tensor_scalar ops)
  - PSUM->SBUF reduction copies ride the ACT engine; memsets ride Pool
All inputs are marshaled host-side into p-major layouts so every DMA is
contiguous per partition.  Each core returns 16 partial sums; the host
combines them into the loss.

This walrus build rejects: gpsimd partition_all_reduce/partition_broadcast,
custom-DVE ops (reciprocal_approx_*), gpsimd TT with broadcast APs or
comparison opcodes, EVENT_SEMAPHORE_RANGE_CLEAR.  Pool (gpsimd) is used
only for dense/strided add/mult/sub/copy/memset.
"""
import sys

for _p in ("/opt/trn_rl_repo",):
    if _p not in sys.path:
        sys.path.insert(0, _p)

import numpy as np

import concourse.bass as bass
import concourse.tile as tile
from concourse import mybir
from concourse.bass_utils import run_bass_kernel_spmd

F32 = mybir.dt.float32
BF16 = mybir.dt.bfloat16
F8 = mybir.dt.float8e4
AX = mybir.AxisListType
OP = mybir.AluOpType
ACTF = mybir.ActivationFunctionType

B, P, C, K = 32, 8732, 81, 16
NCORES = 8
I = B // NCORES          # images per core = 4
PP = 8960                # padded priors = 70 * 128
T = PP // 128            # 70 prior tiles
T8 = 80                  # padded tile count for the m16 max tree
T2 = 72                  # padded tile count for 128-col transpose blocks
NB = T2 * K // 128       # 9 transpose blocks of 128 (t,k)-columns
NCH = 7                  # score chunks (10 tiles each, all 4 images)
CT = T // NCH            # tiles per chunk = 10
THR = 1.0 / 3.0          # ov >= 1/3  <=>  IoU >= 0.5
NQ = 5                   # gathered quantities per box (cx, cy, 5lnw, 5lnh, pad)
TG = 80                  # padded T for the counting grid

# bf16 const blob column offsets (prior rows pre-expanded across K)
B_PX1 = 0
B_PY1 = B_PX1 + T * K
B_PX2 = B_PY1 + T * K
B_PY2 = B_PX2 + T * K
B_TOT = B_PY2 + T * K
# f32 const blob column offsets
C_IV4 = 0
C_PC4 = C_IV4 + T2 * 4
C_ONE = C_PC4 + T2 * 4
C_IDF = C_ONE + 1
C_O128 = C_IDF + 128
C_TOT = C_O128 + 128

_bf16 = np.dtype("uint16")  # bf16 carried as uint16 bit pattern if ml_dtypes absent
try:
    import ml_dtypes

    _bf16 = np.dtype(ml_dtypes.bfloat16)
except ImportError:
    ml_dtypes = None


def _to_bf16(x: np.ndarray) -> np.ndarray:
    if ml_dtypes is not None:
        return x.astype(ml_dtypes.bfloat16)
    u = x.astype(np.float32).view(np.uint32)
    rounded = ((u >> 16) + ((u >> 15) & 1)).astype(np.uint32)
    return (rounded & 0xFFFF).astype(np.uint16)


def _fixup_module(nc: bass.Bass) -> None:
    """Adapt the Tile-generated module to this container's walrus build.

    - EVENT_SEMAPHORE_RANGE_CLEAR is rejected ("ISA wrong length"); the
      preceding Drain(is_reset_sema) already resets the same range, so drop it.
    - Seq-only instructions accept fewer sync waits than Tile emits; hoist
      excess waits onto NoOps placed immediately before (same engine, so
      program order preserves semantics).
    """
    import bass_rust

    for f in nc.m.functions:
        for blk in f.blocks:
            newl = []
            for ins in blk.instructions:
                if getattr(ins, "op_name", None) == "EVENT_SEMAPHORE_RANGE_CLEAR":
                    continue
                si = ins.sync_info
                maxw = 1
                if si is not None and si.on_wait and len(si.on_wait) > maxw:
                    waits = list(si.on_wait)
                    extra, keep = waits[:-maxw], waits[-maxw:]
                    for j in range(0, len(extra), 1):
                        nop = mybir.InstNoOp(
                            name=f"{ins.name}-wsplit{j}", ins=[], outs=[],
                            engine=ins.engine)
                        nop.sync_info = bass_rust.SyncInfo(
                            on_wait=[extra[j]], on_update=[])
                        newl.append(nop)
                    ins.sync_info = bass_rust.SyncInfo(
                        on_wait=keep,
                        on_update=list(si.on_update) if si.on_update else [])
                newl.append(ins)
            blk.instructions = newl


def build_nc(fixup: bool = True) -> bass.Bass:
    nc = bass.Bass()

    # p-major marshaled inputs: [*, 128, free] with contiguous per-partition rows
    d_scores = nc.dram_tensor("scores", [I, 128, T * C], F8, kind="ExternalInput")
    d_locs = nc.dram_tensor("locs", [128, I * T * 4], F32, kind="ExternalInput")
    d_cbf = nc.dram_tensor("cbf", [128, B_TOT], BF16, kind="ExternalInput")
    # counting threshold grid [16, TG]: j/16 (t<T), 999 (pad) -- separate
    # tensor so cbf's readers don't wait for it (tile-granular DMA deps)
    d_ioj = nc.dram_tensor("ioj", [128, 16 * TG], BF16, kind="ExternalInput")
    d_cf32 = nc.dram_tensor("cf32", [128, C_TOT], F32, kind="ExternalInput")
    # all images' box rows (5K each) + onesb row, single partition row
    d_boxall = nc.dram_tensor("boxall", [1, I * 5 * K + 128], BF16,
                              kind="ExternalInput")
    # gather stationaries for all images, p-major
    d_qall = nc.dram_tensor("qall", [128, I * 8 * NQ], BF16,
                            kind="ExternalInput")
    d_lblall = nc.dram_tensor("lblall", [K, I], F32, kind="ExternalInput")
    # 1/(parea + barea) per image, p-major bf16 (host-precomputed)
    d_rA = nc.dram_tensor("rA", [I, 128, T * K], BF16, kind="ExternalInput")
    d_ident = nc.dram_tensor("ident", [128, 128], BF16, kind="ExternalInput")
    d_ones_r = nc.dram_tensor("ones_r", [1, 128], F32, kind="ExternalInput")
    d_iota81 = nc.dram_tensor("iota81", [K, C], F32, kind="ExternalInput")
    # out: per-partition partials; host does the final sums/max.
    # cols: [0:16] scadd (i,4: F, cnt_hi, lps, box), [16:20] ufa (rows 0:K),
    # [20:24] bm, [24:28] np (replicated)
    d_out = nc.dram_tensor("out", [128, 28], F32, kind="ExternalOutput")

    from contextlib import ExitStack

    with tile.TileContext(nc) as tc, ExitStack() as es:
        cpool = es.enter_context(tc.tile_pool(name="consts", bufs=1))
        spool = es.enter_context(tc.tile_pool(name="scores", bufs=1))
        prepool = es.enter_context(tc.tile_pool(name="prein", bufs=1))
        wpool = es.enter_context(tc.tile_pool(name="work", bufs=1))
        epool = es.enter_context(tc.tile_pool(name="exp", bufs=2))
        bpool = es.enter_context(tc.tile_pool(name="batched", bufs=1))
        pp_t = es.enter_context(tc.tile_pool(name="ps_t", bufs=1, space="PSUM"))
        pp_sel = es.enter_context(tc.tile_pool(name="ps_sel", bufs=1, space="PSUM"))
        pp_u = es.enter_context(tc.tile_pool(name="ps_u", bufs=1, space="PSUM"))
        pp_r = es.enter_context(tc.tile_pool(name="ps_r", bufs=2, space="PSUM"))
        rpool = es.enter_context(tc.tile_pool(name="redsb", bufs=4))

        # ------- constants + all per-image loads, DMA-issue spread over
        # ------- SP/ACT queues (engine-synchronous issue serializes)
        cbf_a = cpool.tile([128, 2 * T * K], BF16, tag="cbf_a")
        cbf_b = cpool.tile([128, 2 * T * K], BF16, tag="cbf_b")
        ioj = cpool.tile([128, 16, TG], BF16, tag="ioj")
        cf32 = cpool.tile([128, C_TOT], F32, tag="cf32")
        ident = cpool.tile([128, 128], BF16, tag="ident")
        ones_r = cpool.tile([1, 128], F32, tag="ones_r")
        iota81 = cpool.tile([K, C], F32, tag="iota81")

        sres_all = spool.tile([128, I, T, C], F8, tag="sres")
        rA_l = [prepool.tile([128, T, K], BF16, tag="rA", bufs=2,
                             name=f"rA{i}") for i in range(I)]
        boxall = cpool.tile([1, I * 5 * K + 128], BF16, tag="boxall")
        qall = cpool.tile([128, I, 8 * NQ], BF16, tag="qall")
        lblall = cpool.tile([K, I], F32, tag="lblall")
        l4all = cpool.tile([128, I, T, 4], F32, tag="l4all")
        boxf_l = [boxall[:, i * 5 * K:(i + 1) * 5 * K]
                  .rearrange("p (a k) -> p a k", k=K) for i in range(I)]
        onesb = boxall[:, I * 5 * K:]
        qblk_l = [qall[:, i, :] for i in range(I)]
        lbl_l = [lblall[:, i:i + 1] for i in range(I)]

        # ACT queue: box/onesb row first (bb broadcasts unblock), then rA0
        nc.scalar.dma_start(out=boxall[:], in_=d_boxall[:, :])
        nc.scalar.dma_start(out=rA_l[0][:].rearrange("p t k -> p (t k)"),
                            in_=d_rA[0, :, :])
        # SP queue: jaccard consts first, then everything else by need time
        nc.sync.dma_start(out=cbf_a[:], in_=d_cbf[:, :2 * T * K])
        nc.sync.dma_start(out=cbf_b[:], in_=d_cbf[:, 2 * T * K:])
        nc.sync.dma_start(out=cf32[:], in_=d_cf32[:, :])
        nc.sync.dma_start(out=ones_r[:], in_=d_ones_r[:, :])
        nc.sync.dma_start(out=ident[:], in_=d_ident[:, :])
        nc.sync.dma_start(out=rA_l[1][:].rearrange("p t k -> p (t k)"),
                          in_=d_rA[1, :, :])
        nc.sync.dma_start(out=rA_l[2][:].rearrange("p t k -> p (t k)"),
                          in_=d_rA[2, :, :])
        nc.sync.dma_start(out=rA_l[3][:].rearrange("p t k -> p (t k)"),
                          in_=d_rA[3, :, :])
        nc.sync.dma_start(out=qall[:].rearrange("p i q -> p (i q)"),
                          in_=d_qall[:, :])
        nc.sync.dma_start(out=ioj[:].rearrange("p j t -> p (j t)"),
                          in_=d_ioj[:, :])
        nc.sync.dma_start(
            out=sres_all[:, 0].rearrange("p t c -> p (t c)"),
            in_=d_scores[0, :, :])
        nc.sync.dma_start(
            out=sres_all[:, 1].rearrange("p t c -> p (t c)"),
            in_=d_scores[1, :, :])
        nc.sync.dma_start(
            out=sres_all[:, 2].rearrange("p t c -> p (t c)"),
            in_=d_scores[2, :, :])
        nc.sync.dma_start(
            out=sres_all[:, 3].rearrange("p t c -> p (t c)"),
            in_=d_scores[3, :, :])
        nc.sync.dma_start(out=lblall[:], in_=d_lblall[:, :])
        nc.sync.dma_start(out=l4all[:].rearrange("p i t d -> p (i t d)"),
                          in_=d_locs[:, :])
        nc.sync.dma_start(out=iota81[:], in_=d_iota81[:, :])

        # warm the ACT function table immediately (the implicit
        # ACT_TABLE_LOAD otherwise blocks the first bb copy ~5us later)
        dwarm = rpool.tile([1, 1], F32, tag="dwarm")
        nc.gpsimd.memset(dwarm[:], 0.0)
        nc.scalar.activation(dwarm[:], dwarm[:], ACTF.Relu)

        iotaJ = ioj[:]
        iv4 = cf32[:, C_IV4:C_PC4].rearrange("p (t d) -> p t d", d=4)
        pc4 = cf32[:, C_PC4:C_ONE].rearrange("p (t d) -> p t d", d=4)
        ones_p = cf32[:, C_ONE:C_IDF]
        identf = cf32[:, C_IDF:C_O128]
        ones128 = cf32[:, C_O128:C_TOT]

        def rowsum(dst_row_ap, src_ap, n):
            """[P, n] f32 -> [1, n] partition sum written to dst_row_ap."""
            ps = pp_r.tile([128, 128], F32, tag="red_bc")
            nc.tensor.matmul(ps[0:1, :n], lhsT=ones_p[:src_ap.shape[0], :],
                             rhs=src_ap, start=True, stop=True)
            nc.scalar.copy(dst_row_ap, ps[0:1, :n])

        def bcast_row(dst_ap, row_ap, n):
            """[1, n] f32 -> [128, n] replicated (dst may be bf16)."""
            ps = pp_r.tile([128, 128], F32, tag="red_bc")
            nc.tensor.matmul(ps[:, :n], lhsT=ones_r[:], rhs=row_ap,
                             start=True, stop=True)
            nc.scalar.copy(dst_ap, ps[:, :n])

        def allreduce_sum(dst_ap, src_ap, n):
            ps = pp_r.tile([128, 128], F32, tag="red_bc")
            nc.tensor.matmul(ps[:, :n], lhsT=ones128, rhs=src_ap,
                             start=True, stop=True)
            nc.scalar.copy(dst_ap, ps[:, :n])

        def maxreduce_row(dst_row_ap, src_ap, n):
            """[128, n] f32 -> [1, n] partition max written to dst_row_ap."""
            ps = pp_r.tile([128, 128], F32, tag="red_bc")
            nc.tensor.transpose(ps[:n, :], src_ap, identf)
            tsb = rpool.tile([128, 128], F32, tag="red_tsb")
            nc.scalar.copy(tsb[:n, :], ps[:n, :])
            mx = rpool.tile([128, 1], F32, tag="red_mx")
            nc.vector.tensor_reduce(out=mx[:n, :], in_=tsb[:n, :],
                                    axis=AX.X, op=OP.max)
            ps2 = pp_r.tile([128, 128], F32, tag="red_bc")
            nc.tensor.transpose(ps2[0:1, :n], mx[:n, :], identf[:n, :n])
            nc.scalar.copy(dst_row_ap, ps2[0:1, :n])

        # batched buffers [128, I, ...]
        lse4 = bpool.tile([128, I, T], F32, tag="lse4")
        cen4 = bpool.tile([128, I, T], BF16, tag="cen4")
        pos4 = bpool.tile([128, I, T], F32, tag="pos4")
        out_sb = bpool.tile([128, 28], F32, tag="out_sb")
        np4 = out_sb[:, 24:28]
        npt4 = bpool.tile([128, I], F32, tag="npt4")
        k34 = bpool.tile([128, I], F32, tag="k34")
        cnt_all = bpool.tile([128, I, 16], F32, tag="cnt_all")
        cntr = bpool.tile([128, I, 16], F32, tag="cntr")
        lo4 = bpool.tile([128, I], F32, tag="lo4")
        hi4 = bpool.tile([128, I], F32, tag="hi4")
        scadd = out_sb[:, 0:16].rearrange("p (i s) -> p i s", s=4)
        ufa4 = out_sb[0:K, 16:20]
        bm4 = out_sb[:, 20:24]
        # counting grid scratch (level-1 batched across images; level-2
        # reuses per-image slices of the same tiles)
        cen16 = bpool.tile([128, I, TG], BF16, tag="cen16")
        mask1 = bpool.tile([128, I, 16, TG], BF16, tag="mask1")
        m40a = bpool.tile([128, I, 16, 40], BF16, tag="m40a")
        m20a = bpool.tile([128, I, 16, 20], BF16, tag="m20a")
        m10a = bpool.tile([128, I, 16, 10], BF16, tag="m10a")
        m5a = bpool.tile([128, I, 16, 5], BF16, tag="m5a")
        ge4 = bpool.tile([128, I, 16], F32, tag="ge4")
        mc4 = bpool.tile([128, I], F32, tag="mc4")
        fsc4 = bpool.tile([128, I, T], BF16, tag="fsc4")
        lpsb = bpool.tile([128, I, T], F32, tag="lpsb")
        ce0 = bpool.tile([128, I, T], F32, tag="ce0")

        # grid pads: cen16 pad cols stay 0 (grid pad is 999 -> mask 0)
        nc.gpsimd.memset(cen16[:, :, T:], 0.0)

        # ---------------- per-image box rows via PE broadcast -------------
        bb_l = []
        for i in range(I):
            bbt = prepool.tile([128, 5, K], BF16, tag="bb", bufs=4,
                               name=f"bb{i}")
            ps_bb = pp_r.tile([128, 128], F32, tag="red_bc",
                              name=f"psbb{i}")
            nc.tensor.matmul(ps_bb[:, :5 * K], lhsT=onesb,
                             rhs=boxf_l[i].rearrange("p a k -> p (a k)"),
                             start=True, stop=True)
            nc.scalar.copy(bbt[:].rearrange("p a k -> p (a k)"),
                           ps_bb[:, :5 * K])
            bb_l.append(bbt)

        def emit_J(i):
            """Jaccard + per-box max for image i (DVE-heavy, few stalls)."""
            bb = bb_l[i]

            def bcast_b(row):  # [128, K] box row -> [128, T, K] AP (packed k)
                return bb[:, row, :][:, None, :].broadcast_to([128, T, K])

            # ---------------- jaccard (linear ratio domain, bf16) ---------
            ovp80 = wpool.tile([128, T8, K], BF16, tag="ovp80", bufs=2)
            if i < 2:
                nc.gpsimd.memset(ovp80[:, T:, :], -1.0)
            ov = ovp80[:, :T, :]
            ltxy = wpool.tile([128, 2, T, K], BF16, tag="ltxy")
            w0h0 = wpool.tile([128, 2, T, K], BF16, tag="w0h0")
            wrhr = wpool.tile([128, 2, T, K], BF16, tag="wrhr", bufs=2)
            inter = wpool.tile([128, T, K], BF16, tag="inter")

            # x and y stacked on a free dim: max/min/sub are one op each
            px12 = cbf_a[:].rearrange("p (r t k) -> p r t k", r=2, k=K)
            px34 = cbf_b[:].rearrange("p (r t k) -> p r t k", r=2, k=K)
            b01 = bb[:, 0:2, :][:, :, None, :].broadcast_to([128, 2, T, K])
            b23 = bb[:, 2:4, :][:, :, None, :].broadcast_to([128, 2, T, K])
            nc.vector.tensor_tensor(out=ltxy[:], in0=px12, in1=b01,
                                    op=OP.max)
            nc.vector.tensor_tensor(out=w0h0[:], in0=px34, in1=b23,
                                    op=OP.min)
            nc.vector.tensor_sub(wrhr[:], w0h0[:], ltxy[:])
            nc.scalar.activation(wrhr[:], wrhr[:], ACTF.Relu)
            nc.vector.tensor_mul(inter[:], wrhr[:, 0], wrhr[:, 1])
            nc.vector.tensor_mul(ov, inter[:], rA_l[i][:])

            # per-box max over priors: dense max tree (80 = 2*2*2*2*5),
            # then the cross-partition max (PE transpose round trip)
            tm1 = wpool.tile([128, 40, K], BF16, tag="tm1")
            nc.vector.tensor_tensor(out=tm1[:], in0=ovp80[:, :40, :],
                                    in1=ovp80[:, 40:, :], op=OP.max)
            tm2 = wpool.tile([128, 20, K], BF16, tag="tm2")
            nc.vector.tensor_tensor(out=tm2[:], in0=tm1[:, :20, :],
                                    in1=tm1[:, 20:, :], op=OP.max)
            tm3 = wpool.tile([128, 10, K], BF16, tag="tm3")
            nc.vector.tensor_tensor(out=tm3[:], in0=tm2[:, :10, :],
                                    in1=tm2[:, 10:, :], op=OP.max)
            tm4 = wpool.tile([128, 5, K], BF16, tag="tm4")
            nc.vector.tensor_tensor(out=tm4[:], in0=tm3[:, :5, :],
                                    in1=tm3[:, 5:, :], op=OP.max)
            m16 = wpool.tile([128, K], F32, tag="m16", bufs=2)
            nc.vector.tensor_reduce(
                out=m16[:], in_=tm4[:].rearrange("p t k -> p k t"),
                axis=AX.X, op=OP.max)
            m16row = wpool.tile([1, K], F32, tag="m16row", bufs=2)
            maxreduce_row(m16row[:], m16[:], K)
            return {"ov": ov, "m16row": m16row}

        def emit_F(i, st):
            """Forcing + one-hot + gather + L1 + U for image i."""
            ov = st["ov"]
            l4 = l4all[:, i]
            qblk = qblk_l[i]
            lbl16 = lbl_l[i]
            sres = sres_all[:, i]

            m16rb = wpool.tile([128, K], BF16, tag="m16rb", bufs=2)
            bcast_row(m16rb[:], st["m16row"][:], K)
            fmask = wpool.tile([128, T, K], BF16, tag="fmask")
            nc.vector.tensor_tensor(
                out=fmask[:], in0=ov,
                in1=m16rb[:][:, None, :].broadcast_to([128, T, K]),
                op=OP.is_equal)
            # uniform sentinel 101 (multi-forced priors go multi-hot; rare
            # and bounded): fm2 = fmask*102 - 1 in {-1, 101}
            ovf = wpool.tile([128, T, K], BF16, tag="ovf", bufs=2)
            fm2 = wpool.tile([128, T, K], BF16, tag="fm2")
            nc.vector.tensor_scalar(out=fm2[:], in0=fmask[:],
                                    scalar1=102.0, scalar2=-1.0,
                                    op0=OP.mult, op1=OP.add)
            nc.vector.tensor_tensor(out=ovf[:], in0=fm2[:], in1=ov,
                                    op=OP.max)
            # per-prior max over k: dense tree on the packed innermost dim
            # per-prior max over k: dense tree on the packed innermost dim
            ms1 = wpool.tile([128, T, 8], BF16, tag="ms1")
            nc.vector.tensor_tensor(out=ms1[:], in0=ovf[:, :, 0:8],
                                    in1=ovf[:, :, 8:16], op=OP.max)
            ms2 = wpool.tile([128, T, 4], BF16, tag="ms2")
            nc.vector.tensor_tensor(out=ms2[:], in0=ms1[:, :, 0:4],
                                    in1=ms1[:, :, 4:8], op=OP.max)
            ms3 = wpool.tile([128, T, 2], BF16, tag="ms3")
            nc.vector.tensor_tensor(out=ms3[:], in0=ms2[:, :, 0:2],
                                    in1=ms2[:, :, 2:4], op=OP.max)
            pm = wpool.tile([128, T], BF16, tag="pm")
            nc.vector.tensor_tensor(out=pm[:], in0=ms3[:, :, 0],
                                    in1=ms3[:, :, 1], op=OP.max)
            # pmz = pm where positive else pm+1 (matches nothing): fuses the
            # one-hot and the pos mask into a single is_eq
            pmz = wpool.tile([128, T], BF16, tag="pmz")
            nc.vector.scalar_tensor_tensor(
                out=pmz[:], in0=pm[:], scalar=THR, in1=pm[:],
                op0=OP.is_lt, op1=OP.add)
            # expand pmz across k on ACT so the is_eq runs packed on DVE
            pmze = wpool.tile([128, T, K], BF16, tag="pmze", bufs=2)
            nc.scalar.copy(
                pmze[:], pmz[:][:, :, None].broadcast_to([128, T, K]))
            wm72 = wpool.tile([128, T2 * K], BF16, tag="wm72", bufs=2)
            if i < 2:
                nc.gpsimd.memset(wm72[:, T * K:], 0.0)
            wmat = wm72[:, :T * K].rearrange("p (t k) -> p t k", k=K)
            nc.vector.tensor_tensor(out=wmat, in0=ovf[:], in1=pmze[:],
                                    op=OP.is_equal)
            nc.vector.tensor_scalar(out=pos4[:, i, :], in0=pm[:],
                                    scalar1=THR, scalar2=None,
                                    op0=OP.is_ge, op1=OP.add,
                                    accum_out=npt4[:, i:i + 1])

            # ---------------- box gather via PE ----------------
            ohT_ps = pp_t.tile([128, NB, 128], BF16, tag="ohT")
            for b in range(NB):
                nc.tensor.transpose(
                    ohT_ps[:, b, :],
                    wm72[:, b * 128:(b + 1) * 128],
                    ident[:])
            ohT_sb = wpool.tile([128, NB * 128], BF16, tag="ohT_sb", bufs=2)
            nc.scalar.copy(ohT_sb[:], ohT_ps[:].rearrange("p b n -> p (b n)"))

            sel_ps = pp_sel.tile([8 * NQ, NB, 128], F32, tag="sel")
            for b in range(NB):
                nc.tensor.matmul(sel_ps[:, b, :], lhsT=qblk,
                                 rhs=ohT_sb[:, b * 128:(b + 1) * 128],
                                 start=True, stop=True)
            sel_sb = wpool.tile([8 * NQ, NB * 128], BF16, tag="sel_sb",
                                bufs=2)
            nc.scalar.copy(sel_sb[:], sel_ps[:].rearrange("p b n -> p (b n)"))
            bk_ps = pp_t.tile([128, NB, 8 * NQ], BF16, tag="ohT")
            for b in range(NB):
                nc.tensor.transpose(
                    bk_ps[:, b, :],
                    sel_sb[:, b * 128:(b + 1) * 128],
                    ident[:8 * NQ, :8 * NQ])
            selq = wpool.tile([128, NB * 8 * NQ], BF16, tag="selq", bufs=2)
            nc.scalar.copy(selq[:], bk_ps[:].rearrange("p b n -> p (b n)"))
            # selq[p, (blk*40 + tb*5 + q)] = sel_q at t = blk*8+tb
            sel4 = selq[:].rearrange("p (t q) -> p t q", q=NQ)[:, :, 0:4]

            # ---------------- box L1 (Pool chain + ACT abs-accum) ---------
            lp4 = wpool.tile([128, T, 4], F32, tag="lp4")
            nc.gpsimd.tensor_add(lp4[:], l4, pc4[:, :T, :])
            tb1 = wpool.tile([128, T, 4], F32, tag="tb1")
            nc.gpsimd.tensor_mul(tb1[:], sel4[:, :T, :], iv4[:, :T, :])
            nc.gpsimd.tensor_sub(tb1[:], lp4[:], tb1[:])
            nc.vector.tensor_tensor(
                out=tb1[:], in0=tb1[:],
                in1=pos4[:, i, :][:, :, None].broadcast_to([128, T, 4]),
                op=OP.mult)
            nc.scalar.activation(tb1[:], tb1[:], ACTF.Abs,
                                 accum_out=scadd[:, i, 3:4])

            # ---------------- U matrix (score at label) ----------------
            u_ps = pp_u.tile([K, C], F32, tag="u")
            for t_ in range(T):
                nc.tensor.matmul(u_ps[:], lhsT=wmat[:, t_, :],
                                 rhs=sres[:, t_, :],
                                 start=(t_ == 0), stop=(t_ == T - 1))
            u_sb = wpool.tile([K, C], F32, tag="u_sb", bufs=4)
            nc.scalar.copy(u_sb[:], u_ps[:])
            u_sb_l.append(u_sb)

        # sequential per image: higher cross-engine concurrency (software
        # pipelining J/F) measured ~20% slower per-op from SBUF contention
        u_sb_l = []
        for i in range(I):
            emit_F(i, emit_J(i))

        # ------- CE: exp (ACT) + class sums (bf16 DVE add tree),
        # ------- batched across all I images per chunk -----------------
        for ch in range(NCH):
            et = epool.tile([128, I, CT, C], BF16, tag="et", bufs=3)
            nc.scalar.activation(
                et[:], sres_all[:, :, ch * CT:(ch + 1) * CT, :], ACTF.Exp)
            e3 = et[:].rearrange("p i t c -> p (i t) c")
            t40 = epool.tile([128, I * CT, 40], BF16, tag="t40")
            t20 = epool.tile([128, I * CT, 20], BF16, tag="t20")
            t10 = epool.tile([128, I * CT, 10], BF16, tag="t10")
            t5 = epool.tile([128, I * CT, 5], BF16, tag="t5")
            secc = epool.tile([128, I, CT], F32, tag="secc")
            with nc.allow_low_precision("bf16 class sums"):
                nc.vector.tensor_add(t40[:], e3[:, :, 0:40], e3[:, :, 40:80])
                nc.vector.tensor_add(t20[:], t40[:, :, 0:20],
                                     t40[:, :, 20:40])
                nc.vector.tensor_add(t10[:], t20[:, :, 0:10],
                                     t20[:, :, 10:20])
                nc.vector.tensor_add(t5[:], t10[:, :, 0:5], t10[:, :, 5:10])
            nc.vector.tensor_reduce(
                out=secc[:].rearrange("p i t -> p (i t)"), in_=t5[:],
                axis=AX.X, op=OP.add)
            nc.vector.tensor_add(secc[:], secc[:], et[:, :, :, 80])
            nc.scalar.activation(
                lse4[:, :, ch * CT:(ch + 1) * CT], secc[:], ACTF.Ln)

        # ce0/cen/lps batched
        nc.vector.tensor_sub(ce0[:], lse4[:], sres_all[:, :, :, 0])
        nc.vector.scalar_tensor_tensor(
            out=cen4[:], in0=pos4[:], scalar=0.5,
            in1=ce0[:], op0=OP.is_lt, op1=OP.mult)

        # n_pos allreduce + k3, batched
        allreduce_sum(np4[:], npt4[:], I)
        nc.vector.tensor_scalar(out=k34[:], in0=np4[:], scalar1=3.0,
                                scalar2=None, op0=OP.mult)

        # ---- counting level 1, batched: 16 counts per image via one
        # ---- is_gt against the j/16 grid (cen/16 is exact in bf16)
        nc.vector.tensor_scalar(out=cen16[:, :, :T], in0=cen4[:],
                                scalar1=1.0 / 16, scalar2=None, op0=OP.mult)
        nc.vector.tensor_tensor(
            out=mask1[:],
            in0=cen16[:, :, None, :].broadcast_to([128, I, 16, TG]),
            in1=iotaJ[:, None, :, :].broadcast_to([128, I, 16, TG]),
            op=OP.is_gt)
        with nc.allow_low_precision("bf16 count sums"):
            nc.vector.tensor_add(m40a[:], mask1[:, :, :, 0:40],
                                 mask1[:, :, :, 40:80])
            nc.vector.tensor_add(m20a[:], m40a[:, :, :, 0:20],
                                 m40a[:, :, :, 20:40])
            nc.vector.tensor_add(m10a[:], m20a[:, :, :, 0:10],
                                 m20a[:, :, :, 10:20])
            nc.vector.tensor_add(m5a[:], m10a[:, :, :, 0:5],
                                 m10a[:, :, :, 5:10])
        nc.vector.tensor_reduce(
            out=cnt_all[:].rearrange("p i j -> p (i j)"), in_=m5a[:],
            axis=AX.X, op=OP.add)
        allreduce_sum(cntr[:].rearrange("p i j -> p (i j)"),
                      cnt_all[:].rearrange("p i j -> p (i j)"), I * 16)

        # fill the count-allreduce round trip: lps partial + U-label pick
        nc.vector.tensor_mul(lpsb[:], pos4[:], lse4[:])
        for i in range(I):
            ufx = wpool.tile([K, C], F32, tag="ufx")
            nc.vector.scalar_tensor_tensor(
                out=ufx[:], in0=iota81[:], scalar=lbl_l[i], in1=u_sb_l[i][:],
                op0=OP.is_equal, op1=OP.mult, accum_out=ufa4[:, i:i + 1])

        # ---------------- mining: lo per image, then level-2 batched -----
        for i in range(I):
            # lo = (#edges with count >= k) - 1   (edges j = 0..15)
            nc.vector.tensor_scalar(out=ge4[:, i, :], in0=cntr[:, i, :],
                                    scalar1=k34[:, i:i + 1], scalar2=None,
                                    op0=OP.is_ge, op1=OP.add,
                                    accum_out=lo4[:, i:i + 1])
        nc.vector.tensor_scalar(out=lo4[:], in0=lo4[:],
                                scalar1=-1.0, scalar2=None, op0=OP.add)
        # level 2: thresholds lo + j/16 via (cen - lo) > j/16, all images
        for i in range(I):
            nc.vector.tensor_scalar(out=cen16[:, i, :T],
                                    in0=cen4[:, i, :],
                                    scalar1=lo4[:, i:i + 1],
                                    scalar2=None, op0=OP.subtract)
        nc.vector.tensor_tensor(
            out=mask1[:],
            in0=cen16[:, :, None, :].broadcast_to([128, I, 16, TG]),
            in1=iotaJ[:, None, :, :].broadcast_to([128, I, 16, TG]),
            op=OP.is_gt)
        with nc.allow_low_precision("bf16 count sums"):
            nc.vector.tensor_add(m40a[:], mask1[:, :, :, 0:40],
                                 mask1[:, :, :, 40:80])
            nc.vector.tensor_add(m20a[:], m40a[:, :, :, 0:20],
                                 m40a[:, :, :, 20:40])
            nc.vector.tensor_add(m10a[:], m20a[:, :, :, 0:10],
                                 m20a[:, :, :, 10:20])
            nc.vector.tensor_add(m5a[:], m10a[:, :, :, 0:5],
                                 m10a[:, :, :, 5:10])
        nc.vector.tensor_reduce(
            out=cnt_all[:].rearrange("p i j -> p (i j)"), in_=m5a[:],
            axis=AX.X, op=OP.add)
        allreduce_sum(cntr[:].rearrange("p i j -> p (i j)"),
                      cnt_all[:].rearrange("p i j -> p (i j)"), I * 16)
        nc.vector.tensor_reduce(out=scadd[:, :, 2], in_=lpsb[:],
                                axis=AX.X, op=OP.add)
        for i in range(I):
            nc.vector.tensor_scalar(out=ge4[:, i, :], in0=cntr[:, i, :],
                                    scalar1=k34[:, i:i + 1], scalar2=None,
                                    op0=OP.is_ge, op1=OP.add,
                                    accum_out=mc4[:, i:i + 1])
        nc.vector.tensor_scalar(out=mc4[:], in0=mc4[:], scalar1=1.0 / 16,
                                scalar2=None, op0=OP.mult)
        nc.vector.tensor_add(hi4[:], mc4[:], lo4[:])
        # F(hi), count(hi), boundary max per image (independent chains)
        for i in range(I):
            nc.vector.scalar_tensor_tensor(
                out=fsc4[:, i, :], in0=cen4[:, i, :], scalar=hi4[:, i:i + 1],
                in1=cen4[:, i, :], op0=OP.is_gt, op1=OP.mult,
                accum_out=scadd[:, i, 0:1])
            nc.vector.tensor_scalar(out=fsc4[:, i, :], in0=cen4[:, i, :],
                                    scalar1=hi4[:, i:i + 1], scalar2=None,
                                    op0=OP.is_gt, op1=OP.add,
                                    accum_out=scadd[:, i, 1:2])
            nc.vector.scalar_tensor_tensor(
                out=fsc4[:, i, :], in0=cen4[:, i, :], scalar=hi4[:, i:i + 1],
                in1=cen4[:, i, :], op0=OP.is_le, op1=OP.mult)
            nc.vector.tensor_reduce(out=bm4[:, i:i + 1], in_=fsc4[:, i, :],
                                    axis=AX.X, op=OP.max)

        nc.sync.dma_start(out=d_out[:, :], in_=out_sb[:])

    if fixup:
        _fixup_module(nc)
    return nc


def prepare_inputs(predicted_locs, predicted_scores, boxes, labels,
                   priors_centers):
    """Shard + marshal the full inputs into 8 per-core in_maps (p-major)."""
    predicted_locs = np.asarray(predicted_locs, np.float32)
    predicted_scores = np.asarray(predicted_scores, np.float32)
    boxes = np.asarray(boxes, np.float32)
    labels_f = np.asarray(labels).astype(np.float32)
    priors = np.asarray(priors_centers, np.float32)

    npad = PP - P
    # scores: pad rows have class0=0, others -50 -> lse=0, S0=0, ce0=0 exactly
    pad_scores = np.full((B, npad, C), -50.0, np.float32)
    pad_scores[:, :, 0] = 0.0
    scores_p = np.concatenate([predicted_scores, pad_scores], axis=1)
    # p-major: [B, 128, T*C]
    scores_pm = np.ascontiguousarray(
        scores_p.reshape(B, T, 128, C).transpose(0, 2, 1, 3)
    ).reshape(B, 128, T * C)
    scores_bf = scores_pm.astype(ml_dtypes.float8_e4m3)
    locs_p = np.concatenate(
        [predicted_locs, np.zeros((B, npad, 4), np.float32)], axis=1)
    locs_pm = np.ascontiguousarray(
        locs_p.reshape(B, T, 128, 4).transpose(0, 2, 1, 3)
    ).reshape(B, 128, T * 4)

    # prior rows pre-expanded across K (p-major, bf16)
    pad_pri = np.tile(np.array([-100.0, -100.0, 1.0, 1.0], np.float32),
                      (npad, 1))
    pri = np.concatenate([priors, pad_pri], axis=0)
    pcx, pcy, pw, ph = pri[:, 0], pri[:, 1], pri[:, 2], pri[:, 3]

    def pm_grid(v):  # [PP] -> [128, T]
        return np.ascontiguousarray(v.astype(np.float32).reshape(T, 128).T)

    def pexp(v):  # [PP] -> [128, T*K] expanded across K
        return np.repeat(pm_grid(v)[:, :, None], K, axis=2).reshape(128, T * K)

    # counting threshold grid [16, TG]: j/16 for t<T, 999 on pads
    ioj = np.full((16, TG), 999.0, np.float32)
    ioj[:, :T] = (np.arange(16, dtype=np.float32) / 16.0)[:, None]
    ioj_b = np.tile(ioj.reshape(1, 16 * TG), (128, 1))
    cbf = _to_bf16(np.concatenate(
        [pexp(pcx - pw / 2), pexp(pcy - ph / 2), pexp(pcx + pw / 2),
         pexp(pcy + ph / 2)], axis=1))
    assert cbf.shape[1] == B_TOT
    ioj_bf = _to_bf16(ioj_b)

    # iv4/pc4 [128, T2, 4] host-assembled (d = x, y, w, h; tail zero)
    iv4 = np.zeros((128, T2, 4), np.float32)
    pc4 = np.zeros((128, T2, 4), np.float32)
    iv4[:, :T, 0] = pm_grid(10.0 / pw)
    iv4[:, :T, 1] = pm_grid(10.0 / ph)
    iv4[:, :T, 2] = 1.0
    iv4[:, :T, 3] = 1.0
    pc4[:, :T, 0] = pm_grid(pcx * (10.0 / pw))
    pc4[:, :T, 1] = pm_grid(pcy * (10.0 / ph))
    pc4[:, :T, 2] = pm_grid(5.0 * np.log(pw))
    pc4[:, :T, 3] = pm_grid(5.0 * np.log(ph))

    ones_p = np.ones((128, 1), np.float32)
    identf = np.eye(128, dtype=np.float32)
    cf32 = np.concatenate(
        [iv4.reshape(128, T2 * 4), pc4.reshape(128, T2 * 4),
         ones_p, identf, np.ones((128, 128), np.float32)],
        axis=1).astype(np.float32)
    assert cf32.shape[1] == C_TOT

    bx1, by1, bx2, by2 = (boxes[:, :, d] for d in range(4))
    barea = (bx2 - bx1) * (by2 - by1)
    boxf = np.stack([bx1, by1, bx2, by2, barea], axis=1)      # [B, 5, K]
    # rA[b, p, t, k] = 1/(parea[p,t] + barea[b,k]) (p-major)
    parea_pm = pm_grid(pw * ph)                               # [128, T]
    rA = _to_bf16(1.0 / (
        parea_pm[None, :, :, None] + barea[:, None, None, :].astype(np.float64)
    ).reshape(B, 128, T * K))
    q5 = np.stack([
        (bx1 + bx2) / 2, (by1 + by2) / 2,
        5.0 * np.log(bx2 - bx1), 5.0 * np.log(by2 - by1),
        np.zeros_like(bx1),
    ], axis=2).astype(np.float32)                           # [B, K, 5]
    qblk_f = np.zeros((B, 128, 8 * NQ), np.float32)
    for tb in range(8):
        qblk_f[:, tb * K:(tb + 1) * K, tb * NQ:(tb + 1) * NQ] = q5

    ident = _to_bf16(np.eye(128, dtype=np.float32))
    ones_r = np.ones((1, 128), np.float32)
    iota81 = np.tile(np.arange(C, dtype=np.float32), (K, 1))

    in_maps = []
    for c in range(NCORES):
        sl = slice(c * I, (c + 1) * I)
        boxall = _to_bf16(np.concatenate(
            [boxf[sl].reshape(-1), np.ones(128, np.float32)])[None, :])
        qall = _to_bf16(np.ascontiguousarray(
            qblk_f[sl].transpose(1, 0, 2)).reshape(128, I * 8 * NQ))
        lblall = np.ascontiguousarray(labels_f[sl].T)         # [K, I]
        locs2 = np.ascontiguousarray(
            locs_pm[sl].transpose(1, 0, 2)).reshape(128, I * T * 4)
        in_maps.append({
            "scores": scores_bf[sl],
            "locs": locs2,
            "cbf": cbf,
            "ioj": ioj_bf,
            "cf32": cf32,
            "boxall": boxall,
            "qall": qall,
            "lblall": lblall,
            "rA": rA[sl],
            "ident": ident,
            "ones_r": ones_r,
            "iota81": iota81,
        })
    return in_maps


def combine_outputs(outs):
    """outs: list of 8 per-core [128, 28] partials -> scalar loss."""
    n_pos_total = 0.0
    box_sum = 0.0
    class_sum = 0.0
    for o in outs:
        o = np.asarray(o, np.float64)
        sc = o[:, :16].reshape(128, I, 4).sum(axis=0)   # [I, 4]
        uf = o[:K, 16:20].sum(axis=0)                   # [I]
        bm = o[:, 20:24].max(axis=0)                    # [I]
        npi = o[0, 24:28]                               # [I]
        f_sum, cnt_hi, lps, box = sc[:, 0], sc[:, 1], sc[:, 2], sc[:, 3]
        r = (3.0 * npi - cnt_hi) * bm + f_sum
        cep = lps - uf
        n_pos_total += npi.sum()
        box_sum += box.sum()
        class_sum += (cep + r).sum()
    loss = class_sum / n_pos_total + box_sum / (n_pos_total * 4.0)
    return np.float32(loss)


_NC_CACHE = {}


def kernel(predicted_locs, predicted_scores, boxes, labels, priors_centers):
    if "nc" not in _NC_CACHE:
        _NC_CACHE["nc"] = build_nc()
    nc = _NC_CACHE["nc"]
    in_maps = prepare_inputs(predicted_locs, predicted_scores, boxes, labels,
                             priors_centers)
    res = run_bass_kernel_spmd(nc, in_maps, list(range(NCORES)))
    outs = [res.results[c]["out"] for c in range(NCORES)]
    return combine_outputs(outs)


if __name__ == "__main__":
    import reference as R

    inputs = {k: np.asarray(v) for k, v in R.setup_inputs().items()}
    print("loss =", kernel(**inputs))
